# revision 1
# baseline (speedup 1.0000x reference)
"""Trainium2 Bass kernel for nn_DecoderWithAttention (Show-Attend-Tell decoder).

Strategy (8 NeuronCores, tensor-parallel):
 - Gate/hidden dims of both LSTMs, attention dim A, feature dim F (for awe),
   and vocab V are sharded 8 ways. Batch (128) stays whole on every core and
   is the SBUF partition dim.
 - All weights live resident in SBUF as bf16 (pre-transposed on host); all
   matmuls are bf16 x bf16 -> f32 PSUM. Elementwise/cell/softmax math is f32.
 - Recurrent state h1/h2 is kept TRANSPOSED ([d, b]) because every consumer
   (gate matmuls, attention, FC) wants it as the stationary lhsT operand.
   Each core computes its 128-wide slice of h, transposes it on the PE, and
   the slices are exchanged with AllGather collectives (4 per step: h1T,
   e-partials, aweT, h2T).
 - Per-step constant gate input U[t] = feats_mean @ W1b.T + emb_t @ W1c.T +
   biases is precomputed on device (teacher forcing makes emb_t known).
 - att1 = feats @ Wf.T is precomputed on device in transposed layout
   [a, (n, b)], A-sharded per core.
 - awe = einsum('bnf,bn->bf') runs on the PE as 36 accumulating matmuls with
   diag(alpha_n) as the stationary operand (diag built by DVE from eye*alpha).
 - The decode-length masking of the reference only affects outputs (frozen
   states never feed an active output), so the recurrence runs unmasked and
   `active` multiplies the logits only.
 - FC (logits) for step t runs inside step t+1's collective gaps; output is
   V-sharded and assembled on host.

Host side: stable argsort by length (the reference returns the SORTED batch
order), embedding gather, transposes/casts to bf16, weight slicing per core.
"""
import sys, os
sys.path.insert(0, "/opt/trn_rl_repo")

import numpy as np
import ml_dtypes

BF = ml_dtypes.bfloat16

# problem dims (hardcoded per the task contract)
B, N, F, A, E, D, V, L = 128, 36, 2048, 1024, 1024, 1024, 10000, 20
T = L - 1                       # 19 decode steps
NC = 8                          # cores
DS = D // NC                    # 128   hidden slice
GS = 4 * DS                     # 512   gate slice (i,f,g,o blocks of DS)
FS = F // NC                    # 256   feature slice (awe)
VS = V // NC                    # 1250  vocab slice
KD = D // 128                   # 8     k-tiles over D
KF = F // 128                   # 16    k-tiles over F
NB = N * B                      # 4608  (n, b) flattened
NCHUNK = 9                      # e/att1T chunks over n (4 n's per chunk)
CW = NB // NCHUNK               # 512   chunk width

_PROG = None  # cached (nc, input_names) build


def _build():
    from concourse import bass, tile, mybir, bacc

    dt = mybir.dt
    nc = bacc.Bacc("TRN2", target_bir_lowering=False, debug=False,
                   num_devices=NC)

    def din(name, shape, d=dt.bfloat16):
        return nc.dram_tensor(name, shape, d, kind="ExternalInput").ap()

    # ---- inputs (per-core unless noted shared) ----
    featsT = din("featsT", [F, NB])            # shared  [f, (n, b)]
    embsT = din("embsT", [T * E, B])           # shared  [(t, e), b]
    fmeanT = din("fmeanT", [F, B])             # shared  [f, b]
    eye = din("eye", [128, 128])               # shared  identity / diag mask
    actm = din("actm", [B, T], dt.float32)     # shared  active mask
    featsaw = din("featsaw", [B, N * FS])      # per-core feats f-slice [b,(n,fs)]
    w1aT = din("w1aT", [D, GS])                # W1_ih[rows, :D].T      (h2 block)
    w1hT = din("w1hT", [D, GS])                # W1_hh[rows].T
    w1bT = din("w1bT", [F, GS])                # W1_ih[rows, D:D+F].T   (fmean)
    w1cT = din("w1cT", [E, GS])                # W1_ih[rows, D+F:].T    (emb)
    w2aT = din("w2aT", [F, GS])                # W2_ih[rows, :F].T      (awe)
    w2bT = din("w2bT", [D, GS])                # W2_ih[rows, F:].T      (h1)
    w2hT = din("w2hT", [D, GS])                # W2_hh[rows].T
    wdT = din("wdT", [D, DS])                  # Wd[a_slice].T
    wfT = din("wfT", [F, DS])                  # Wf[a_slice].T
    wacol = din("wacol", [DS, 1])              # Wa[0, a_slice] column
    wfcT = din("wfcT", [D, VS])                # Wfc[v_slice].T
    bg1 = din("bg1", [1, GS])                  # (b1_ih+b1_hh)[rows]
    bg2 = din("bg2", [1, GS])                  # (b2_ih+b2_hh)[rows]
    batt = din("batt", [1, DS])                # (bf+bd)[a_slice]
    bfc = din("bfc", [1, VS])                  # bfc[v_slice]

    preds_o = nc.dram_tensor("preds", [T * B, VS], dt.float32,
                             kind="ExternalOutput").ap()

    AG = mybir.AluOpType.bypass
    AF = mybir.ActivationFunctionType
    OP = mybir.AluOpType
    AX = mybir.AxisListType
    RG = [list(range(NC))]

    with tile.TileContext(nc) as tc:
        with tc.tile_pool(name="kw", bufs=1) as kw, \
             tc.tile_pool(name="kst", bufs=1) as kst, \
             tc.tile_pool(name="pre", bufs=1) as pre, \
             tc.tile_pool(name="ld", bufs=2) as ld, \
             tc.tile_pool(name="wrk", bufs=3) as wrk, \
             tc.tile_pool(name="cell", bufs=2) as cellp, \
             tc.tile_pool(name="wrk2", bufs=2) as wrk2, \
             tc.tile_pool(name="pfb", bufs=1) as pfb, \
             tc.tile_pool(name="pg", bufs=2, space="PSUM") as pg, \
             tc.tile_pool(name="pmix", bufs=3, space="PSUM") as pmix, \
             tc.tile_pool(name="pfc", bufs=1, space="PSUM") as pfc, \
             tc.tile_pool(name="dram", bufs=1, space="DRAM") as dram:

            bf16 = dt.bfloat16
            f32 = dt.float32

            # ---------- resident loads ----------
            def load(pool, src, shape, tag):
                t = pool.tile(shape, bf16, tag=tag)
                nc.sync.dma_start(t[:], src[:].rearrange(
                    "(k p) m -> p k m", p=128) if len(shape) == 3 else src[:])
                return t

            # weights stored [128, ktiles, width]
            w1aT_s = load(kw, w1aT, [128, KD, GS], "w1aT")
            w1hT_s = load(kw, w1hT, [128, KD, GS], "w1hT")
            w2aT_s = load(kw, w2aT, [128, KF, GS], "w2aT")
            w2bT_s = load(kw, w2bT, [128, KD, GS], "w2bT")
            w2hT_s = load(kw, w2hT, [128, KD, GS], "w2hT")
            wdT_s = load(kw, wdT, [128, KD, DS], "wdT")
            wfcT_s = load(kw, wfcT, [128, KD, VS], "wfcT")
            wacol_s = load(kw, wacol, [128, 1], "wacol")
            eye_s = load(kw, eye, [128, 128], "eye")
            featsaw_s = kw.tile([128, N, FS], bf16, tag="featsaw")
            nc.sync.dma_start(featsaw_s[:], featsaw[:].rearrange("b (n s) -> b n s", n=N))
            actm_s = kw.tile([128, T], f32, tag="actm")
            nc.sync.dma_start(actm_s[:], actm[:])
            bg2_s = kw.tile([1, GS], bf16, tag="bg2")
            nc.sync.dma_start(bg2_s[:], bg2[:])
            bfc_s = kw.tile([1, VS], bf16, tag="bfc")
            nc.sync.dma_start(bfc_s[:], bfc[:])

            ones_s = kw.tile([1, CW], bf16, tag="ones")
            nc.vector.memset(ones_s[:], 1.0)

            # persistent state / gathered tensors
            att1T_s = kst.tile([128, NB], bf16, tag="att1T")      # [a, (n,b)] slice
            uc_s = kst.tile([128, T, GS], bf16, tag="uc")         # U[t] gate const
            h1T_s = kst.tile([128, KD, 128], bf16, tag="h1T")     # gathered h1T
            h2T_s = kst.tile([128, KD, 128], bf16, tag="h2T")     # gathered h2T
            aweT_s = kst.tile([128, KF, 128], bf16, tag="aweT")   # gathered aweT
            c1_s = kst.tile([128, DS], f32, tag="c1")
            c2_s = kst.tile([128, DS], f32, tag="c2")
            nc.vector.memset(c1_s[:], 0.0)
            nc.vector.memset(c2_s[:], 0.0)

            # DRAM bounce buffers for the collectives
            ag1_in = dram.tile([128, 128], bf16, tag="ag1i")
            ag1_out = dram.tile([NC * 128, 128], bf16, tag="ag1o")
            ag2_in = dram.tile([1, NB], bf16, tag="ag2i")
            ag2_out = dram.tile([1, NB], bf16, tag="ag2o")
            ag3_in = dram.tile([FS, 128], bf16, tag="ag3i")
            ag3_out = dram.tile([NC * FS, 128], bf16, tag="ag3o")
            ag4_in = dram.tile([128, 128], bf16, tag="ag4i")
            ag4_out = dram.tile([NC * 128, 128], bf16, tag="ag4o")

            # ---------- precompute: U1 (fmean + bias) ----------
            w1cT_s = pre.tile([128, KD, GS], bf16, tag="w1cT")
            nc.sync.dma_start(w1cT_s[:], w1cT[:].rearrange("(k p) m -> p k m", p=128))
            wfT_s = pre.tile([128, KF, DS], bf16, tag="wfT")
            nc.sync.dma_start(wfT_s[:], wfT[:].rearrange("(k p) m -> p k m", p=128))
            bg1_s = pre.tile([1, GS], bf16, tag="bg1")
            nc.sync.dma_start(bg1_s[:], bg1[:])
            batt_s = pre.tile([1, DS], bf16, tag="batt")
            nc.sync.dma_start(batt_s[:], batt[:])
            u1_sb = pre.tile([128, GS], f32, tag="u1")

            u1_ps = pg.tile([128, GS], f32, tag="pg")
            for k in range(KF):
                fm = ld.tile([128, 128], bf16, tag="fmch")
                nc.sync.dma_start(fm[:], fmeanT[k * 128:(k + 1) * 128, :])
                wb = ld.tile([128, GS], bf16, tag="wbch")
                nc.sync.dma_start(wb[:], w1bT[k * 128:(k + 1) * 128, :])
                nc.tensor.matmul(u1_ps[:], fm[:], wb[:],
                                 start=(k == 0), stop=False)
            nc.tensor.matmul(u1_ps[:], ones_s[0:1, 0:128], bg1_s[:],
                             start=False, stop=True)
            nc.vector.tensor_copy(u1_sb[:], u1_ps[:])

            # ---------- precompute: Uemb[t] (emitted t=0 now, rest later) ----
            def emit_uemb(t):
                et = ld.tile([128, KD, 128], bf16, tag="embt")
                nc.sync.dma_start(
                    et[:], embsT[t * E:(t + 1) * E, :].rearrange(
                        "(k p) m -> p k m", p=128))
                ue_ps = pg.tile([128, GS], f32, tag="pg")
                for k in range(KD):
                    nc.tensor.matmul(ue_ps[:], et[:, k, :], w1cT_s[:, k, :],
                                     start=(k == 0), stop=(k == KD - 1))
                nc.vector.tensor_tensor(uc_s[:, t, :], ue_ps[:], u1_sb[:], OP.add)

            emit_uemb(0)

            # ---------- precompute: att1T (A-sliced, [a, (b, n)]) ----------
            for cg in range(3):  # column groups of 1536 (3 psum chunks each)
                a1_pss = []
                for _cc in range(3):
                    a1c = pmix.tile([128, CW], f32, tag="pmix")
                    a1_pss.append(a1c)
                for k in range(KF):
                    fch = ld.tile([128, 3 * CW], bf16, tag="fch")
                    nc.scalar.dma_start(
                        fch[:], featsT[k * 128:(k + 1) * 128,
                                       cg * 3 * CW:(cg + 1) * 3 * CW])
                    for cc in range(3):
                        nc.tensor.matmul(
                            a1_pss[cc][:], wfT_s[:, k, :],
                            fch[:, cc * CW:(cc + 1) * CW],
                            start=(k == 0), stop=False)
                for cc in range(3):
                    c = cg * 3 + cc
                    nc.tensor.matmul(a1_pss[cc][:], batt_s[:],
                                     ones_s[0:1, 0:CW],
                                     start=False, stop=True)
                    nc.vector.tensor_copy(att1T_s[:, c * CW:(c + 1) * CW],
                                          a1_pss[cc][:])

            for t in range(1, 4):
                emit_uemb(t)

            # ---------- step loop ----------
            for t in range(T):
                # --- LSTM1 gates ---
                g1_ps = pg.tile([128, GS], f32, tag="pg")
                g1_sb = wrk2.tile([128, GS], f32, tag="gsb")
                if t > 0:
                    for k in range(KD):
                        nc.tensor.matmul(g1_ps[:], h2T_s[:, k, :],
                                         w1aT_s[:, k, :], start=(k == 0),
                                         stop=False)
                    for k in range(KD):
                        nc.tensor.matmul(g1_ps[:], h1T_s[:, k, :],
                                         w1hT_s[:, k, :], start=False,
                                         stop=(k == KD - 1))
                    nc.vector.tensor_tensor(g1_sb[:], g1_ps[:], uc_s[:, t, :],
                                            OP.add)
                else:
                    nc.vector.tensor_copy(g1_sb[:], uc_s[:, 0, :])

                # --- g2 psum opens early: h2-block + bias run in the AG1 gap
                g2_ps = pg.tile([128, GS], f32, tag="pg")
                nc.tensor.matmul(g2_ps[:], ones_s[0:1, 0:128], bg2_s[:],
                                 start=True, stop=False)
                if t > 0:
                    for k in range(KD):
                        nc.tensor.matmul(g2_ps[:], h2T_s[:, k, :],
                                         w2hT_s[:, k, :], start=False,
                                         stop=False)

                # --- cell 1 -> h1 (f32) , h1 bf16, h1T ---
                h1_bf = _cell(nc, tc, cellp, wrk, g1_sb, c1_s, AF, OP)
                h1T_ps = pmix.tile([128, 128], bf16, tag="pmix")
                nc.tensor.transpose(h1T_ps[:], h1_bf[:], eye_s[:])
                h1T_loc = wrk.tile([128, 128], bf16, tag="hTloc")
                nc.vector.tensor_copy(h1T_loc[:], h1T_ps[:])

                # --- AG1: h1T ---
                nc.sync.dma_start(ag1_in[:], h1T_loc[:])
                nc.gpsimd.collective_compute(
                    "AllGather", AG, replica_groups=RG,
                    ins=[ag1_in.opt()], outs=[ag1_out.opt()])
                nc.scalar.dma_start(h1T_s[:], ag1_out[:].rearrange(
                    "(k p) m -> p k m", p=128))

                # --- att2T = Wd_slice @ h1 (transposed out [a, b]) ---
                at2_ps = pmix.tile([128, 128], f32, tag="pmix")
                for k in range(KD):
                    nc.tensor.matmul(at2_ps[:], wdT_s[:, k, :], h1T_s[:, k, :],
                                     start=(k == 0), stop=(k == KD - 1))
                at2_bf = wrk.tile([128, 128], bf16, tag="at2")
                nc.vector.tensor_copy(at2_bf[:], at2_ps[:])

                # --- g2 h1-block (ready now; fills DVE rt-add time on PE) ---
                for k in range(KD):
                    nc.tensor.matmul(g2_ps[:], h1T_s[:, k, :], w2bT_s[:, k, :],
                                     start=False, stop=False)

                # --- e chunks (b-major): rT = relu(att1T + att2T); e = Wa . rT
                #     att1T is stored [a, (b, n)] so e partials come out
                #     b-major and the post-AllReduce load is contiguous.
                rt = kst.tile([128, 128, N], bf16, tag="rt")
                for bc in range(4):
                    js = slice(bc * 32, (bc + 1) * 32)
                    nc.vector.tensor_tensor(
                        rt[:, js, :],
                        att1T_s[:, bc * 32 * N:(bc + 1) * 32 * N].rearrange(
                            "p (j n) -> p j n", n=N),
                        at2_bf[:, js].rearrange("p (j o) -> p j o", o=1)
                        .broadcast_to((128, 32, N)), OP.add)
                    nc.vector.tensor_scalar_max(
                        rt[:, js, :], rt[:, js, :], 0.0)
                rtf = rt[:].rearrange("p j n -> p (j n)")
                for c in range(NCHUNK):
                    e_ps = pmix.tile([1, CW], f32, tag="pmix")
                    nc.tensor.matmul(e_ps[:], wacol_s[:],
                                     rtf[:, c * CW:(c + 1) * CW],
                                     start=True, stop=True)
                    e_row = wrk2.tile([1, CW], bf16, tag="erow")
                    nc.scalar.copy(e_row[:], e_ps[:])
                    eng = nc.sync if c % 2 == 0 else nc.scalar
                    eng.dma_start(ag2_in[:, c * CW:(c + 1) * CW], e_row[:])

                # --- AR2: sum e partials across cores (CCE add) ---
                nc.gpsimd.collective_compute(
                    "AllReduce", OP.add, replica_groups=RG,
                    ins=[ag2_in.opt()], outs=[ag2_out.opt()])

                # --- FC for step t-1 + deferred Uemb (fill the AR2 gap) ---
                if t > 0:
                    _emit_fc(nc, t - 1, pfc, pfb, h2T_s, wfcT_s, ones_s,
                             bfc_s, actm_s, preds_o, KD, VS, f32)
                if 4 + t < T:
                    emit_uemb(4 + t)
                e_sb = wrk.tile([128, N], bf16, tag="esb")
                nc.sync.dma_start(e_sb[:], ag2_out[:].rearrange(
                    "o (b n) -> (o b) n", n=N))
                emax = wrk.tile([128, 1], f32, tag="emax")
                nc.vector.tensor_reduce(emax[:], e_sb[:], AX.X, OP.max,
                                        negate=True)
                expo = wrk.tile([128, N], f32, tag="expo")
                nc.scalar.activation(expo[:], e_sb[:], AF.Exp, bias=emax[:])
                esum = wrk.tile([128, 1], f32, tag="esum")
                nc.vector.tensor_reduce(esum[:], expo[:], AX.X, OP.add)
                erec = wrk.tile([128, 1], f32, tag="erec")
                nc.vector.reciprocal(erec[:], esum[:])
                alpha_bf = wrk.tile([128, N], bf16, tag="alpha")
                nc.vector.tensor_scalar_mul(alpha_bf[:], expo[:], erec[:])

                # --- awe: 36 diag matmuls; out [b, fs] ---
                awe_ps = pmix.tile([128, FS], f32, tag="pmix")
                eye_b = eye_s[:].rearrange("p (o j) -> p o j", o=1) \
                    .broadcast_to((128, 4, 128))
                for gi in range(9):
                    dch = wrk.tile([128, 4, 128], bf16, tag="dch")
                    nc.vector.tensor_tensor(
                        dch[:], eye_b,
                        alpha_bf[:, gi * 4:(gi + 1) * 4].rearrange(
                            "p (n o) -> p n o", o=1).broadcast_to((128, 4, 128)),
                        OP.mult)
                    for j in range(4):
                        n = gi * 4 + j
                        nc.tensor.matmul(awe_ps[:], dch[:, j, :],
                                         featsaw_s[:, n, :],
                                         start=(n == 0), stop=(n == N - 1))
                awe_bf = wrk.tile([128, FS], bf16, tag="awebf")
                nc.vector.tensor_copy(awe_bf[:], awe_ps[:])
                for h in range(FS // 128):
                    awT_ps = pmix.tile([128, 128], bf16, tag="pmix")
                    nc.tensor.transpose(awT_ps[:],
                                        awe_bf[:, h * 128:(h + 1) * 128],
                                        eye_s[:])
                    awT_sb = wrk.tile([128, 128], bf16, tag="awTsb")
                    nc.vector.tensor_copy(awT_sb[:], awT_ps[:])
                    nc.sync.dma_start(ag3_in[h * 128:(h + 1) * 128, :],
                                      awT_sb[:])

                # --- AG3: aweT ---
                nc.gpsimd.collective_compute(
                    "AllGather", AG, replica_groups=RG,
                    ins=[ag3_in.opt()], outs=[ag3_out.opt()])
                nc.scalar.dma_start(aweT_s[:], ag3_out[:].rearrange(
                    "(k p) m -> p k m", p=128))

                # --- LSTM2 gates: awe-block closes the accumulation ---
                for k in range(KF):
                    nc.tensor.matmul(g2_ps[:], aweT_s[:, k, :], w2aT_s[:, k, :],
                                     start=False, stop=(k == KF - 1))
                g2_sb = wrk2.tile([128, GS], f32, tag="gsb")
                nc.vector.tensor_copy(g2_sb[:], g2_ps[:])

                # --- cell 2 -> h2, h2T, AG4 ---
                h2_bf = _cell(nc, tc, cellp, wrk, g2_sb, c2_s, AF, OP)
                h2T_ps = pmix.tile([128, 128], bf16, tag="pmix")
                nc.tensor.transpose(h2T_ps[:], h2_bf[:], eye_s[:])
                h2T_loc = wrk.tile([128, 128], bf16, tag="hTloc")
                nc.vector.tensor_copy(h2T_loc[:], h2T_ps[:])
                nc.sync.dma_start(ag4_in[:], h2T_loc[:])
                nc.gpsimd.collective_compute(
                    "AllGather", AG, replica_groups=RG,
                    ins=[ag4_in.opt()], outs=[ag4_out.opt()])
                nc.scalar.dma_start(h2T_s[:], ag4_out[:].rearrange(
                    "(k p) m -> p k m", p=128))

            # final FC for last step
            _emit_fc(nc, T - 1, pfc, pfb, h2T_s, wfcT_s, ones_s, bfc_s,
                     actm_s, preds_o, KD, VS, f32)

    nc.compile()
    return nc


def _cell(nc, tc, cellp, wrk, g_sb, c_s, AF, OP):
    """LSTM cell elementwise: gates [128, 512] f32 -> h bf16 [128,128].
    Updates c_s in place."""
    from concourse import mybir
    bf16 = mybir.dt.bfloat16
    f32 = mybir.dt.float32
    i_s = cellp.tile([128, DS], f32, tag="ci")
    nc.scalar.activation(i_s[:], g_sb[:, 0:DS], AF.Sigmoid)
    f_s = cellp.tile([128, DS], f32, tag="cf")
    nc.scalar.activation(f_s[:], g_sb[:, DS:2 * DS], AF.Sigmoid)
    t_g = cellp.tile([128, DS], f32, tag="cg")
    nc.scalar.activation(t_g[:], g_sb[:, 2 * DS:3 * DS], AF.Tanh)
    o_s = cellp.tile([128, DS], f32, tag="co")
    nc.scalar.activation(o_s[:], g_sb[:, 3 * DS:4 * DS], AF.Sigmoid)
    t1 = wrk.tile([128, DS], f32, tag="t1")
    nc.vector.tensor_tensor(t1[:], f_s[:], c_s[:], OP.mult)
    t2 = wrk.tile([128, DS], f32, tag="t2")
    nc.vector.tensor_tensor(t2[:], i_s[:], t_g[:], OP.mult)
    nc.vector.tensor_tensor(c_s[:], t1[:], t2[:], OP.add)
    tc2 = wrk.tile([128, DS], f32, tag="tc2")
    nc.scalar.activation(tc2[:], c_s[:], AF.Tanh)
    h_bf = wrk.tile([128, DS], bf16, tag="hbf")
    nc.vector.tensor_tensor(h_bf[:], o_s[:], tc2[:], OP.mult)
    return h_bf


def _emit_fc(nc, t, pfc, pfb, h2T_s, wfcT_s, ones_s, bfc_s, actm_s,
             preds_o, KD, VS, f32):
    """logits for step t: [128, VS] = h2(t) @ WfcT + bfc, masked by active."""
    fc_ps = pfc.tile([128, VS], f32, tag="pfc")
    p_sb = pfb.tile([128, VS], f32, tag="psb")
    for lo in range(0, VS, 512):
        hi = min(lo + 512, VS)
        for k in range(KD):
            nc.tensor.matmul(fc_ps[:, lo:hi], h2T_s[:, k, :],
                             wfcT_s[:, k, lo:hi], start=(k == 0), stop=False)
        nc.tensor.matmul(fc_ps[:, lo:hi], ones_s[0:1, 0:128], bfc_s[:, lo:hi],
                         start=False, stop=True)
        nc.vector.tensor_scalar_mul(p_sb[:, lo:hi], fc_ps[:, lo:hi],
                                    actm_s[:, t:t + 1])
    nc.sync.dma_start(preds_o[t * B:(t + 1) * B, :], p_sb[:])


def _host_prep(inputs):
    """Sort, gather, transpose, cast, slice per core."""
    f32 = np.float32
    lengths = np.asarray(inputs["caption_lengths"])[:, 0]
    sort_ind = np.argsort(-lengths, kind="stable")
    feats = np.asarray(inputs["image_features"], f32)[sort_ind]        # [B,N,F]
    caps = np.asarray(inputs["encoded_captions"])[sort_ind]            # [B,L]
    dec_len = lengths[sort_ind] - 1
    emb = np.asarray(inputs["emb"], f32)
    embs = emb[caps[:, :T]]                                            # [B,T,E]
    fmean = feats.mean(axis=1)                                         # [B,F]

    featsT = np.ascontiguousarray(feats.transpose(2, 0, 1)).reshape(F, NB)
    embsT = np.ascontiguousarray(embs.transpose(1, 2, 0)).reshape(T * E, B)
    fmeanT = np.ascontiguousarray(fmean.T)                             # [F,B]
    actm = (np.arange(T)[None, :] < dec_len[:, None]).astype(f32)      # [B,T]
    eye = np.eye(128, dtype=BF)

    W1 = np.asarray(inputs["W1_ih"], f32); W1h = np.asarray(inputs["W1_hh"], f32)
    W2 = np.asarray(inputs["W2_ih"], f32); W2h = np.asarray(inputs["W2_hh"], f32)
    Wf = np.asarray(inputs["Wf"], f32); Wd = np.asarray(inputs["Wd"], f32)
    Wa = np.asarray(inputs["Wa"], f32); Wfc = np.asarray(inputs["Wfc"], f32)
    b1 = np.asarray(inputs["b1_ih"], f32) + np.asarray(inputs["b1_hh"], f32)
    b2 = np.asarray(inputs["b2_ih"], f32) + np.asarray(inputs["b2_hh"], f32)
    bfv = np.asarray(inputs["bf"], f32) + np.asarray(inputs["bd"], f32)
    bfc = np.asarray(inputs["bfc"], f32)

    shared = {
        "featsT": featsT.astype(BF), "embsT": embsT.astype(BF),
        "fmeanT": fmeanT.astype(BF), "eye": eye, "actm": actm,
    }
    tp = lambda x: np.ascontiguousarray(x.T).astype(BF)
    in_maps = []
    for i in range(NC):
        rows = np.concatenate([np.arange(q * D + i * DS, q * D + (i + 1) * DS)
                               for q in range(4)])
        asl = slice(i * DS, (i + 1) * DS)
        m = dict(shared)
        m["featsaw"] = np.ascontiguousarray(
            feats[:, :, i * FS:(i + 1) * FS]).reshape(B, N * FS).astype(BF)
        m["w1aT"] = tp(W1[rows, 0:D])
        m["w1bT"] = tp(W1[rows, D:D + F])
        m["w1cT"] = tp(W1[rows, D + F:])
        m["w1hT"] = tp(W1h[rows])
        m["w2aT"] = tp(W2[rows, 0:F])
        m["w2bT"] = tp(W2[rows, F:])
        m["w2hT"] = tp(W2h[rows])
        m["wdT"] = tp(Wd[asl])
        m["wfT"] = tp(Wf[asl])
        m["wacol"] = np.ascontiguousarray(Wa[0, asl])[:, None].astype(BF)
        m["wfcT"] = tp(Wfc[i * VS:(i + 1) * VS])
        m["bg1"] = b1[rows][None, :].astype(BF)
        m["bg2"] = b2[rows][None, :].astype(BF)
        m["batt"] = bfv[asl][None, :].astype(BF)
        m["bfc"] = bfc[i * VS:(i + 1) * VS][None, :].astype(BF)
        in_maps.append(m)
    return in_maps


def kernel(**inputs):
    global _PROG
    from concourse.bass_utils import run_bass_kernel_spmd
    if _PROG is None:
        _PROG = _build()
    in_maps = _host_prep(inputs)
    res = run_bass_kernel_spmd(
        _PROG, in_maps, core_ids=list(range(NC)),
        trace=os.environ.get("KERNEL_TRACE") == "1")
    if res.exec_time_ns is not None:
        kernel.last_exec_time_ns = res.exec_time_ns
    preds = np.concatenate(
        [res.results[i]["preds"].reshape(T, B, VS) for i in range(NC)], axis=2)
    return np.ascontiguousarray(preds.transpose(1, 0, 2))



# revision 6
# speedup vs baseline: 1.1475x; 1.1475x over previous
"""Trainium2 Bass kernel for nn_DecoderWithAttention (Show-Attend-Tell decoder).

v2 strategy (8 NeuronCores, tensor-parallel, 3 collectives/step):
 - Gate/hidden dims of both LSTMs, attention dim A, and vocab V sharded 8
   ways; batch B=128 whole on every core as the partition dim.
 - awe is never materialized: since g2_awe = awe @ W2a.T with
   awe = sum_n alpha_n * feats_n, we precompute M_n = feats[:,n,:] @ W2a.T
   (per-core gate slice) once, and per step accumulate
   g2 += sum_n diag(alpha[:,n]) @ M_n directly in PSUM (36 matmuls).
   This removes the aweT AllGather (AG3), its transposes, and featsaw.
 - Per step: AG1 (h1T 32KB), AR2 (e-partials 9KB), AG4 (h2T 32KB); all
   collective outputs in Shared DRAM address space.
 - U[t] (emb/fmean gate constant) and biases are injected into PSUM via
   identity/ones matmuls; LSTM cells read gates straight from PSUM.
 - FC (logits, V-sharded) is split in two chunks filling the AG1 and AR2
   gaps; Uemb prefetch also fills the AR2 gap; next-step g1 h1-block +
   U-load + g2 bias fill the AG4 gap.
 - att1T/featsT/e use n-major layout [a, (n, b)] so the e AllReduce output
   loads directly as [b, n] for the replicated softmax.

Host side: stable argsort by length (reference returns the sorted batch
order), embedding gather, transposes/casts to bf16, weight slicing per core.
"""
import sys, os
sys.path.insert(0, "/opt/trn_rl_repo")

import numpy as np
import ml_dtypes

BF = ml_dtypes.bfloat16

# problem dims (hardcoded per the task contract)
B, N, F, A, E, D, V, L = 128, 36, 2048, 1024, 1024, 1024, 10000, 20
T = L - 1                       # 19 decode steps
NC = 8                          # cores
DS = D // NC                    # 128   hidden slice
GS = 4 * DS                     # 512   gate slice (i,f,g,o blocks of DS)
VS = V // NC                    # 1250  vocab slice
VH = 625                        # FC chunk width (2 chunks)
KD = D // 128                   # 8     k-tiles over D
KF = F // 128                   # 16    k-tiles over F
NB = N * B                      # 4608  (n, b) flattened
NCHUNK = 9                      # e chunks over (n, b)
CW = NB // NCHUNK               # 512   chunk width (4 n's)

_PROG = None  # cached build


def _build():
    from concourse import bass, tile, mybir, bacc

    dt = mybir.dt
    nc = bacc.Bacc("TRN2", target_bir_lowering=False, debug=False,
                   num_devices=NC)

    def din(name, shape, d=dt.bfloat16):
        return nc.dram_tensor(name, shape, d, kind="ExternalInput").ap()

    # ---- inputs (per-core unless noted shared) ----
    featsT = din("featsT", [F, NB])            # shared  [f, (n, b)]
    embsT = din("embsT", [T * E, B])           # shared  [(t, e), b]
    fmeanT = din("fmeanT", [F, B])             # shared  [f, b]
    eye = din("eye", [128, 128])               # shared  identity
    actm = din("actm", [B, T], dt.float32)     # shared  active mask
    w1aT = din("w1aT", [D, GS])                # W1_ih[rows, :D].T      (h2 block)
    w1hT = din("w1hT", [D, GS])                # W1_hh[rows].T
    w1bT = din("w1bT", [F, GS])                # W1_ih[rows, D:D+F].T   (fmean)
    w1cT = din("w1cT", [E, GS])                # W1_ih[rows, D+F:].T    (emb)
    w2aT = din("w2aT", [F, GS])                # W2_ih[rows, :F].T      (awe)
    w2bT = din("w2bT", [D, GS])                # W2_ih[rows, F:].T      (h1)
    w2hT = din("w2hT", [D, GS])                # W2_hh[rows].T
    wdT = din("wdT", [D, DS])                  # Wd[a_slice].T
    wfT = din("wfT", [F, DS])                  # Wf[a_slice].T
    wacol = din("wacol", [DS, 1])              # Wa[0, a_slice] column
    wfcT = din("wfcT", [D, VS])                # Wfc[v_slice].T
    bg1 = din("bg1", [1, GS])                  # (b1_ih+b1_hh)[rows]
    bg2 = din("bg2", [1, GS])                  # (b2_ih+b2_hh)[rows]
    batt = din("batt", [1, DS])                # (bf+bd)[a_slice]
    bfc = din("bfc", [1, VS])                  # bfc[v_slice]

    preds_o = nc.dram_tensor("preds", [T * B, VS], dt.float32,
                             kind="ExternalOutput").ap()

    AG = mybir.AluOpType.bypass
    AF = mybir.ActivationFunctionType
    OP = mybir.AluOpType
    AX = mybir.AxisListType
    RG = [list(range(NC))]

    with tile.TileContext(nc) as tc:
        with tc.tile_pool(name="kw", bufs=1) as kw, \
             tc.tile_pool(name="kst", bufs=1) as kst, \
             tc.tile_pool(name="pre", bufs=1) as pre, \
             tc.tile_pool(name="ld", bufs=2) as ld, \
             tc.tile_pool(name="wrk", bufs=2) as wrk, \
             tc.tile_pool(name="cell", bufs=1) as cellp, \
             tc.tile_pool(name="wrk2", bufs=2) as wrk2, \
             tc.tile_pool(name="pfb", bufs=1) as pfb, \
             tc.tile_pool(name="pg", bufs=3, space="PSUM") as pg, \
             tc.tile_pool(name="pmix", bufs=3, space="PSUM") as pmix, \
             tc.tile_pool(name="pfc", bufs=1, space="PSUM") as pfc, \
             tc.tile_pool(name="dram", bufs=1, space="DRAM") as dram:

            bf16 = dt.bfloat16
            f32 = dt.float32

            # ---------- resident loads ----------
            def load(pool, src, shape, tag):
                t = pool.tile(shape, bf16, tag=tag)
                nc.sync.dma_start(t[:], src[:].rearrange(
                    "(k p) m -> p k m", p=128) if len(shape) == 3 else src[:])
                return t

            # weights stored [128, ktiles, width]
            w1aT_s = load(kw, w1aT, [128, KD, GS], "w1aT")
            w1hT_s = load(kw, w1hT, [128, KD, GS], "w1hT")
            w2aT_s = load(kw, w2aT, [128, KF, GS], "w2aT")
            w2bT_s = load(kw, w2bT, [128, KD, GS], "w2bT")
            w2hT_s = load(kw, w2hT, [128, KD, GS], "w2hT")
            wdT_s = load(kw, wdT, [128, KD, DS], "wdT")
            wfcT_s = load(kw, wfcT, [128, KD, VS], "wfcT")
            wacol_s = load(kw, wacol, [128, 1], "wacol")
            eye_s = load(kw, eye, [128, 128], "eye")
            actm_s = kw.tile([128, T], f32, tag="actm")
            nc.sync.dma_start(actm_s[:], actm[:])
            bg2_s = kw.tile([1, GS], bf16, tag="bg2")
            nc.sync.dma_start(bg2_s[:], bg2[:])
            bfc_s = kw.tile([1, VS], bf16, tag="bfc")
            nc.sync.dma_start(bfc_s[:], bfc[:])

            ones_s = kw.tile([1, CW], bf16, tag="ones")
            nc.vector.memset(ones_s[:], 1.0)

            # persistent state / gathered tensors
            att1T_s = kst.tile([128, NB], bf16, tag="att1T")      # [a, (n,b)]
            uc_s = kst.tile([128, 6, GS], bf16, tag="uc")         # U[t] rotating
            m_s = kst.tile([128, N, GS], bf16, tag="m_s")         # M_n  [b,(n,gs)]
            h1T_s = kst.tile([128, KD, 128], bf16, tag="h1T")     # gathered h1T
            h2T_s = kst.tile([128, KD, 128], bf16, tag="h2T")     # gathered h2T
            dch_s = kst.tile([128, N, 128], bf16, tag="dch")      # diag(alpha_n)
            c1_s = kst.tile([128, DS], f32, tag="c1")
            c2_s = kst.tile([128, DS], f32, tag="c2")
            nc.vector.memset(c1_s[:], 0.0)
            nc.vector.memset(c2_s[:], 0.0)

            # DRAM bounce buffers; collective outputs in Shared space
            ag1_in = dram.tile([128, 128], bf16, tag="ag1i")
            ag2_in = dram.tile([1, NB], bf16, tag="ag2i")
            ag4_in = dram.tile([128, 128], bf16, tag="ag4i")
            ag1_out = nc.dram_tensor("ag1o", [NC * 128, 128], bf16,
                                     addr_space="Shared").ap()
            ag2_out = nc.dram_tensor("ag2o", [1, NB], bf16,
                                     addr_space="Shared").ap()
            ag4_out = nc.dram_tensor("ag4o", [NC * 128, 128], bf16,
                                     addr_space="Shared").ap()

            # ---------- precompute: U1 (fmean + bias) ----------
            w1cT_s = pre.tile([128, KD, GS], bf16, tag="w1cT")
            nc.sync.dma_start(w1cT_s[:], w1cT[:].rearrange("(k p) m -> p k m", p=128))
            wfT_s = pre.tile([128, KF, DS], bf16, tag="wfT")
            nc.sync.dma_start(wfT_s[:], wfT[:].rearrange("(k p) m -> p k m", p=128))
            bg1_s = pre.tile([1, GS], bf16, tag="bg1")
            nc.sync.dma_start(bg1_s[:], bg1[:])
            batt_s = pre.tile([1, DS], bf16, tag="batt")
            nc.sync.dma_start(batt_s[:], batt[:])
            u1_sb = pre.tile([128, GS], f32, tag="u1")

            u1_ps = pg.tile([128, GS], f32, tag="pg")
            for k in range(KF):
                fm = ld.tile([128, 128], bf16, tag="fmch")
                nc.sync.dma_start(fm[:], fmeanT[k * 128:(k + 1) * 128, :])
                wb = ld.tile([128, GS], bf16, tag="wbch")
                nc.sync.dma_start(wb[:], w1bT[k * 128:(k + 1) * 128, :])
                nc.tensor.matmul(u1_ps[:], fm[:], wb[:],
                                 start=(k == 0), stop=False)
            nc.tensor.matmul(u1_ps[:], ones_s[0:1, 0:128], bg1_s[:],
                             start=False, stop=True)
            nc.vector.tensor_copy(u1_sb[:], u1_ps[:])

            # ---------- precompute: Uemb[t] (t=0..3 now, rest in-loop) ----
            def emit_uemb(t):
                et = ld.tile([128, KD, 128], bf16, tag="embt")
                nc.sync.dma_start(
                    et[:], embsT[t * E:(t + 1) * E, :].rearrange(
                        "(k p) m -> p k m", p=128))
                ue_ps = pg.tile([128, GS], f32, tag="pg")
                for k in range(KD):
                    nc.tensor.matmul(ue_ps[:], et[:, k, :], w1cT_s[:, k, :],
                                     start=(k == 0), stop=(k == KD - 1))
                nc.vector.tensor_tensor(uc_s[:, t % 6, :], ue_ps[:], u1_sb[:], OP.add)

            emit_uemb(0)

            # ---------- precompute: att1T ([a, (n, b)], A-sliced) ----------
            for cg in range(3):  # column groups of 1536 (3 psum chunks each)
                a1_pss = []
                for _cc in range(3):
                    a1c = pmix.tile([128, CW], f32, tag="pmix")
                    a1_pss.append(a1c)
                for k in range(KF):
                    fch = ld.tile([128, 3 * CW], bf16, tag="fch")
                    nc.scalar.dma_start(
                        fch[:], featsT[k * 128:(k + 1) * 128,
                                       cg * 3 * CW:(cg + 1) * 3 * CW])
                    for cc in range(3):
                        nc.tensor.matmul(
                            a1_pss[cc][:], wfT_s[:, k, :],
                            fch[:, cc * CW:(cc + 1) * CW],
                            start=(k == 0), stop=False)
                for cc in range(3):
                    c = cg * 3 + cc
                    nc.tensor.matmul(a1_pss[cc][:], batt_s[:],
                                     ones_s[0:1, 0:CW],
                                     start=False, stop=True)
                    nc.vector.tensor_copy(att1T_s[:, c * CW:(c + 1) * CW],
                                          a1_pss[cc][:])

            # ---------- precompute: M_n = feats_n @ W2a_slice.T ----------
            for n in range(N):
                ftile = ld.tile([128, KF, 128], bf16, tag="ftile")
                nc.sync.dma_start(
                    ftile[:], featsT[:, n * 128:(n + 1) * 128].rearrange(
                        "(k p) m -> p k m", p=128))
                m_ps = pg.tile([128, GS], f32, tag="pg")
                for k in range(KF):
                    nc.tensor.matmul(m_ps[:], ftile[:, k, :], w2aT_s[:, k, :],
                                     start=(k == 0), stop=(k == KF - 1))
                eng = nc.vector if n % 2 == 0 else nc.scalar
                if n % 2 == 0:
                    nc.vector.tensor_copy(m_s[:, n, :], m_ps[:])
                else:
                    nc.scalar.copy(m_s[:, n, :], m_ps[:])

            for t in range(1, 4):
                emit_uemb(t)

            # ---------- step loop ----------
            g1_ps = None
            g2_ps = None

            def open_gates(t):
                """g1 = U[t] (+ h1T(t-1) block), g2 = bias; emitted during
                the AG4(t-1) flight (h1T(t-1) is already gathered). The g1
                group is closed here for t==0 (no h2 block follows)."""
                g1 = pg.tile([128, GS], f32, tag="pg")
                nc.tensor.matmul(g1[:], eye_s[:], uc_s[:, t % 6, :],
                                 start=True, stop=(t == 0))
                if t > 0:
                    for k in range(KD):
                        nc.tensor.matmul(g1[:], h1T_s[:, k, :],
                                         w1hT_s[:, k, :], start=False,
                                         stop=False)
                g2 = pg.tile([128, GS], f32, tag="pg")
                nc.tensor.matmul(g2[:], ones_s[0:1, 0:128], bg2_s[:],
                                 start=True, stop=False)
                return g1, g2

            g1_ps, g2_ps = open_gates(0)

            for t in range(T):
                # --- post-AG4(t-1): h2 blocks of g1 and g2 ---
                if t > 0:
                    for k in range(KD):
                        nc.tensor.matmul(g1_ps[:], h2T_s[:, k, :],
                                         w1aT_s[:, k, :], start=False,
                                         stop=(k == KD - 1))
                    for k in range(KD):
                        nc.tensor.matmul(g2_ps[:], h2T_s[:, k, :],
                                         w2hT_s[:, k, :], start=False,
                                         stop=False)

                # --- cell 1 -> h1 bf16, transpose, AG1 ---
                h1_bf = _cell(nc, tc, cellp, wrk, g1_ps, c1_s, AF, OP)
                h1T_ps = pmix.tile([128, 128], bf16, tag="pmix")
                nc.tensor.transpose(h1T_ps[:], h1_bf[:], eye_s[:])
                h1T_loc = wrk.tile([128, 128], bf16, tag="hTloc")
                nc.vector.tensor_copy(h1T_loc[:], h1T_ps[:])
                nc.sync.dma_start(ag1_in[:], h1T_loc[:])
                nc.gpsimd.collective_compute(
                    "AllGather", AG, replica_groups=RG,
                    ins=[ag1_in.opt()], outs=[ag1_out.opt()])

                # --- AG1 gap: FC(t-1) chunk 0 ---
                if t > 0:
                    _emit_fc(nc, t - 1, 0, pfc, pfb, h2T_s, wfcT_s, ones_s,
                             bfc_s, actm_s, preds_o, KD, f32)
                nc.scalar.dma_start(h1T_s[:], ag1_out[:].rearrange(
                    "(k p) m -> p k m", p=128))

                # --- att2T = Wd_slice @ h1 ([a, b]) ---
                at2_ps = pmix.tile([128, 128], f32, tag="pmix")
                for k in range(KD):
                    nc.tensor.matmul(at2_ps[:], wdT_s[:, k, :], h1T_s[:, k, :],
                                     start=(k == 0), stop=(k == KD - 1))
                at2_bf = wrk.tile([128, 128], bf16, tag="at2")
                nc.vector.tensor_copy(at2_bf[:], at2_ps[:])

                # --- rt = relu(att1T + at2T) in [a, (n, b)]; e chunks ---
                rt = kst.tile([128, N, 128], bf16, tag="rt")
                for nh in range(2):
                    ns = slice(nh * 18, (nh + 1) * 18)
                    nc.vector.tensor_tensor(
                        rt[:, ns, :],
                        att1T_s[:, nh * 18 * 128:(nh + 1) * 18 * 128]
                        .rearrange("p (n b) -> p n b", b=128),
                        at2_bf[:].rearrange("p (o b) -> p o b", o=1)
                        .broadcast_to((128, 18, 128)), OP.add)
                    nc.vector.tensor_scalar_max(
                        rt[:, ns, :], rt[:, ns, :], 0.0)
                rtf = rt[:].rearrange("p n b -> p (n b)")
                for c in range(NCHUNK):
                    e_ps = pmix.tile([1, CW], f32, tag="pmix")
                    nc.tensor.matmul(e_ps[:], wacol_s[:],
                                     rtf[:, c * CW:(c + 1) * CW],
                                     start=True, stop=True)
                    e_row = wrk2.tile([1, CW], bf16, tag="erow")
                    nc.scalar.copy(e_row[:], e_ps[:])
                    eng = nc.sync if c % 2 == 0 else nc.scalar
                    eng.dma_start(ag2_in[:, c * CW:(c + 1) * CW], e_row[:])

                # --- AR2: sum e partials across cores ---
                nc.gpsimd.collective_compute(
                    "AllReduce", OP.add, replica_groups=RG,
                    ins=[ag2_in.opt()], outs=[ag2_out.opt()])

                # --- AR2 gap: g2 h1-block, FC(t-1) chunk 1, Uemb(t+4) ---
                for k in range(KD):
                    nc.tensor.matmul(g2_ps[:], h1T_s[:, k, :], w2bT_s[:, k, :],
                                     start=False, stop=False)
                if t > 0:
                    _emit_fc(nc, t - 1, 1, pfc, pfb, h2T_s, wfcT_s, ones_s,
                             bfc_s, actm_s, preds_o, KD, f32)
                if 4 + t < T:
                    emit_uemb(4 + t)

                # --- softmax (replicated, [b, n]) ---
                e_sb = wrk.tile([128, N], bf16, tag="esb")
                nc.sync.dma_start(e_sb[:], ag2_out[:].rearrange(
                    "o (n b) -> (o b) n", b=128))
                emax = wrk.tile([128, 1], f32, tag="emax")
                nc.vector.tensor_reduce(emax[:], e_sb[:], AX.X, OP.max,
                                        negate=True)
                expo = wrk.tile([128, N], f32, tag="expo")
                nc.scalar.activation(expo[:], e_sb[:], AF.Exp, bias=emax[:])
                esum = wrk.tile([128, 1], f32, tag="esum")
                nc.vector.tensor_reduce(esum[:], expo[:], AX.X, OP.add)
                erec = wrk.tile([128, 1], f32, tag="erec")
                nc.vector.reciprocal(erec[:], esum[:])
                alpha_bf = wrk.tile([128, N], bf16, tag="alpha")
                nc.vector.tensor_scalar_mul(alpha_bf[:], expo[:], erec[:])

                # --- diag(alpha_n) builds + g2 += sum_n diag_n @ M_n ---
                eye_b = eye_s[:].rearrange("p (o b) -> p o b", o=1)
                for nh in range(2):
                    ns = slice(nh * 18, (nh + 1) * 18)
                    nc.vector.tensor_tensor(
                        dch_s[:, ns, :],
                        eye_b.broadcast_to((128, 18, 128)),
                        alpha_bf[:, ns].rearrange("p (n o) -> p n o", o=1)
                        .broadcast_to((128, 18, 128)), OP.mult)
                    for n in range(nh * 18, (nh + 1) * 18):
                        nc.tensor.matmul(g2_ps[:], dch_s[:, n, :],
                                         m_s[:, n, :], start=False,
                                         stop=(n == N - 1))

                # --- cell 2 -> h2, transpose, AG4 ---
                h2_bf = _cell(nc, tc, cellp, wrk, g2_ps, c2_s, AF, OP)
                h2T_ps = pmix.tile([128, 128], bf16, tag="pmix")
                nc.tensor.transpose(h2T_ps[:], h2_bf[:], eye_s[:])
                h2T_loc = wrk.tile([128, 128], bf16, tag="hTloc")
                nc.vector.tensor_copy(h2T_loc[:], h2T_ps[:])
                nc.sync.dma_start(ag4_in[:], h2T_loc[:])
                nc.gpsimd.collective_compute(
                    "AllGather", AG, replica_groups=RG,
                    ins=[ag4_in.opt()], outs=[ag4_out.opt()])

                # --- AG4 gap: next step's g1 U/h1 blocks + g2 bias ---
                if t + 1 < T:
                    g1_ps, g2_ps = open_gates(t + 1)
                nc.scalar.dma_start(h2T_s[:], ag4_out[:].rearrange(
                    "(k p) m -> p k m", p=128))

            # final FC for last step
            _emit_fc(nc, T - 1, 0, pfc, pfb, h2T_s, wfcT_s, ones_s, bfc_s,
                     actm_s, preds_o, KD, f32)
            _emit_fc(nc, T - 1, 1, pfc, pfb, h2T_s, wfcT_s, ones_s, bfc_s,
                     actm_s, preds_o, KD, f32)

    nc.compile()
    return nc


def _cell(nc, tc, cellp, wrk, g_ps, c_s, AF, OP):
    """LSTM cell elementwise: gates [128, 512] f32 PSUM -> h bf16 [128,128].
    Updates c_s in place. Gate layout: [i, f, g, o] blocks of DS."""
    from concourse import mybir
    bf16 = mybir.dt.bfloat16
    f32 = mybir.dt.float32
    i_s = cellp.tile([128, 2 * DS], f32, tag="cif")
    nc.scalar.activation(i_s[:], g_ps[:, 0:2 * DS], AF.Sigmoid)
    t_g = cellp.tile([128, DS], f32, tag="cg")
    nc.scalar.activation(t_g[:], g_ps[:, 2 * DS:3 * DS], AF.Tanh)
    o_s = cellp.tile([128, DS], f32, tag="co")
    nc.scalar.activation(o_s[:], g_ps[:, 3 * DS:4 * DS], AF.Sigmoid)
    t1 = wrk.tile([128, DS], f32, tag="t1")
    nc.vector.tensor_tensor(t1[:], i_s[:, DS:2 * DS], c_s[:], OP.mult)
    t2 = wrk.tile([128, DS], f32, tag="t2")
    nc.vector.tensor_tensor(t2[:], i_s[:, 0:DS], t_g[:], OP.mult)
    nc.vector.tensor_tensor(c_s[:], t1[:], t2[:], OP.add)
    tc2 = wrk.tile([128, DS], f32, tag="tc2")
    nc.scalar.activation(tc2[:], c_s[:], AF.Tanh)
    h_bf = wrk.tile([128, DS], bf16, tag="hbf")
    nc.vector.tensor_tensor(h_bf[:], o_s[:], tc2[:], OP.mult)
    return h_bf


def _emit_fc(nc, t, half, pfc, pfb, h2T_s, wfcT_s, ones_s, bfc_s, actm_s,
             preds_o, KD, f32):
    """logits chunk for step t: [128, VH] = h2(t) @ WfcT[:, lo:hi] + bfc,
    masked by active."""
    lo0, hi0 = half * VH, (half + 1) * VH
    fc_ps = pfc.tile([128, VH], f32, tag="pfc")
    p_sb = pfb.tile([128, VH], f32, tag="psb")
    for lo in range(lo0, hi0, 512):
        hi = min(lo + 512, hi0)
        for k in range(KD):
            nc.tensor.matmul(fc_ps[:, lo - lo0:hi - lo0], h2T_s[:, k, :],
                             wfcT_s[:, k, lo:hi], start=(k == 0), stop=False)
        nc.tensor.matmul(fc_ps[:, lo - lo0:hi - lo0], ones_s[0:1, 0:128],
                         bfc_s[:, lo:hi], start=False, stop=True)
    nc.vector.tensor_scalar_mul(p_sb[:], fc_ps[:], actm_s[:, t:t + 1])
    nc.sync.dma_start(preds_o[t * B:(t + 1) * B, lo0:hi0], p_sb[:])


def _host_prep(inputs):
    """Sort, gather, transpose, cast, slice per core."""
    f32 = np.float32
    lengths = np.asarray(inputs["caption_lengths"])[:, 0]
    sort_ind = np.argsort(-lengths, kind="stable")
    feats = np.asarray(inputs["image_features"], f32)[sort_ind]        # [B,N,F]
    caps = np.asarray(inputs["encoded_captions"])[sort_ind]            # [B,L]
    dec_len = lengths[sort_ind] - 1
    emb = np.asarray(inputs["emb"], f32)
    embs = emb[caps[:, :T]]                                            # [B,T,E]
    fmean = feats.mean(axis=1)                                         # [B,F]

    # n-major: featsT[f, (n, b)]
    featsT = np.ascontiguousarray(feats.transpose(2, 1, 0)).reshape(F, NB)
    embsT = np.ascontiguousarray(embs.transpose(1, 2, 0)).reshape(T * E, B)
    fmeanT = np.ascontiguousarray(fmean.T)                             # [F,B]
    actm = (np.arange(T)[None, :] < dec_len[:, None]).astype(f32)      # [B,T]
    eye = np.eye(128, dtype=BF)

    W1 = np.asarray(inputs["W1_ih"], f32); W1h = np.asarray(inputs["W1_hh"], f32)
    W2 = np.asarray(inputs["W2_ih"], f32); W2h = np.asarray(inputs["W2_hh"], f32)
    Wf = np.asarray(inputs["Wf"], f32); Wd = np.asarray(inputs["Wd"], f32)
    Wa = np.asarray(inputs["Wa"], f32); Wfc = np.asarray(inputs["Wfc"], f32)
    b1 = np.asarray(inputs["b1_ih"], f32) + np.asarray(inputs["b1_hh"], f32)
    b2 = np.asarray(inputs["b2_ih"], f32) + np.asarray(inputs["b2_hh"], f32)
    bfv = np.asarray(inputs["bf"], f32) + np.asarray(inputs["bd"], f32)
    bfc = np.asarray(inputs["bfc"], f32)

    shared = {
        "featsT": featsT.astype(BF), "embsT": embsT.astype(BF),
        "fmeanT": fmeanT.astype(BF), "eye": eye, "actm": actm,
    }
    tp = lambda x: np.ascontiguousarray(x.T).astype(BF)
    in_maps = []
    for i in range(NC):
        rows = np.concatenate([np.arange(q * D + i * DS, q * D + (i + 1) * DS)
                               for q in range(4)])
        asl = slice(i * DS, (i + 1) * DS)
        m = dict(shared)
        m["w1aT"] = tp(W1[rows, 0:D])
        m["w1bT"] = tp(W1[rows, D:D + F])
        m["w1cT"] = tp(W1[rows, D + F:])
        m["w1hT"] = tp(W1h[rows])
        m["w2aT"] = tp(W2[rows, 0:F])
        m["w2bT"] = tp(W2[rows, F:])
        m["w2hT"] = tp(W2h[rows])
        m["wdT"] = tp(Wd[asl])
        m["wfT"] = tp(Wf[asl])
        m["wacol"] = np.ascontiguousarray(Wa[0, asl])[:, None].astype(BF)
        m["wfcT"] = tp(Wfc[i * VS:(i + 1) * VS])
        m["bg1"] = b1[rows][None, :].astype(BF)
        m["bg2"] = b2[rows][None, :].astype(BF)
        m["batt"] = bfv[asl][None, :].astype(BF)
        m["bfc"] = bfc[i * VS:(i + 1) * VS][None, :].astype(BF)
        in_maps.append(m)
    return in_maps


def kernel(**inputs):
    global _PROG
    from concourse.bass_utils import run_bass_kernel_spmd
    if _PROG is None:
        _PROG = _build()
    in_maps = _host_prep(inputs)
    res = run_bass_kernel_spmd(
        _PROG, in_maps, core_ids=list(range(NC)),
        trace=os.environ.get("KERNEL_TRACE") == "1")
    if res.exec_time_ns is not None:
        kernel.last_exec_time_ns = res.exec_time_ns
    preds = np.concatenate(
        [res.results[i]["preds"].reshape(T, B, VS) for i in range(NC)], axis=2)
    return np.ascontiguousarray(preds.transpose(1, 0, 2))


# revision 9
# speedup vs baseline: 1.2407x; 1.0813x over previous
"""Trainium2 Bass kernel for nn_DecoderWithAttention (Show-Attend-Tell decoder).

v2 strategy (8 NeuronCores, tensor-parallel, 3 collectives/step):
 - Gate/hidden dims of both LSTMs, attention dim A, and vocab V sharded 8
   ways; batch B=128 whole on every core as the partition dim.
 - awe is never materialized: since g2_awe = awe @ W2a.T with
   awe = sum_n alpha_n * feats_n, we precompute M_n = feats[:,n,:] @ W2a.T
   (per-core gate slice) once, and per step accumulate
   g2 += sum_n diag(alpha[:,n]) @ M_n directly in PSUM (36 matmuls).
   This removes the aweT AllGather (AG3), its transposes, and featsaw.
 - Per step: AG1 (h1T 32KB), AR2 (e-partials 9KB), AG4 (h2T 32KB); all
   collective outputs in Shared DRAM address space.
 - U[t] (emb/fmean gate constant) and biases are injected into PSUM via
   identity/ones matmuls; LSTM cells read gates straight from PSUM.
 - FC (logits, V-sharded) is split in two chunks filling the AG1 and AR2
   gaps; Uemb prefetch also fills the AR2 gap; next-step g1 h1-block +
   U-load + g2 bias fill the AG4 gap.
 - att1T/featsT/e use b-major layout [a, (b, n)] so the e AllReduce output
   loads as [b, n] with contiguous per-partition DMA segments.

Host side: stable argsort by length (reference returns the sorted batch
order), embedding gather, transposes/casts to bf16, weight slicing per core.
"""
import sys, os
sys.path.insert(0, "/opt/trn_rl_repo")

import numpy as np
import ml_dtypes

BF = ml_dtypes.bfloat16

# problem dims (hardcoded per the task contract)
B, N, F, A, E, D, V, L = 128, 36, 2048, 1024, 1024, 1024, 10000, 20
T = L - 1                       # 19 decode steps
NC = 8                          # cores
DS = D // NC                    # 128   hidden slice
GS = 4 * DS                     # 512   gate slice (i,f,g,o blocks of DS)
VS = V // NC                    # 1250  vocab slice
VH = 625                        # FC chunk width (2 chunks)
KD = D // 128                   # 8     k-tiles over D
KF = F // 128                   # 16    k-tiles over F
NB = N * B                      # 4608  (n, b) flattened
NCHUNK = 9                      # e chunks over (n, b)
CW = NB // NCHUNK               # 512   chunk width (4 n's)

_PROG = None  # cached build


def _build():
    from concourse import bass, tile, mybir, bacc

    dt = mybir.dt
    nc = bacc.Bacc("TRN2", target_bir_lowering=False, debug=False,
                   num_devices=NC)

    def din(name, shape, d=dt.bfloat16):
        return nc.dram_tensor(name, shape, d, kind="ExternalInput").ap()

    # ---- inputs (per-core unless noted shared) ----
    featsT = din("featsT", [F, NB])            # shared  [f, (b, n)]
    featsN = din("featsN", [F, NB])            # shared  [f, (n, b)]
    embsT = din("embsT", [T * E, B])           # shared  [(t, e), b]
    fmeanT = din("fmeanT", [F, B])             # shared  [f, b]
    eye = din("eye", [128, 128])               # shared  identity
    actm = din("actm", [B, T], dt.float32)     # shared  active mask
    w1aT = din("w1aT", [D, GS])                # W1_ih[rows, :D].T      (h2 block)
    w1hT = din("w1hT", [D, GS])                # W1_hh[rows].T
    w1bT = din("w1bT", [F, GS])                # W1_ih[rows, D:D+F].T   (fmean)
    w1cT = din("w1cT", [E, GS])                # W1_ih[rows, D+F:].T    (emb)
    w2aT = din("w2aT", [F, GS])                # W2_ih[rows, :F].T      (awe)
    w2bT = din("w2bT", [D, GS])                # W2_ih[rows, F:].T      (h1)
    w2hT = din("w2hT", [D, GS])                # W2_hh[rows].T
    wdT = din("wdT", [D, DS])                  # Wd[a_slice].T
    wfT = din("wfT", [F, DS])                  # Wf[a_slice].T
    wacol = din("wacol", [DS, 1])              # Wa[0, a_slice] column
    wfcT = din("wfcT", [D, VS])                # Wfc[v_slice].T
    bg1 = din("bg1", [1, GS])                  # (b1_ih+b1_hh)[rows]
    bg2 = din("bg2", [1, GS])                  # (b2_ih+b2_hh)[rows]
    batt = din("batt", [1, DS])                # (bf+bd)[a_slice]
    bfc = din("bfc", [1, VS])                  # bfc[v_slice]

    preds_o = nc.dram_tensor("preds", [T * B, VS], dt.float32,
                             kind="ExternalOutput").ap()

    AG = mybir.AluOpType.bypass
    AF = mybir.ActivationFunctionType
    OP = mybir.AluOpType
    AX = mybir.AxisListType
    RG = [list(range(NC))]

    with tile.TileContext(nc) as tc:
        with tc.tile_pool(name="kw", bufs=1) as kw, \
             tc.tile_pool(name="kst", bufs=1) as kst, \
             tc.tile_pool(name="pre", bufs=1) as pre, \
             tc.tile_pool(name="ld", bufs=2) as ld, \
             tc.tile_pool(name="wrk", bufs=2) as wrk, \
             tc.tile_pool(name="cell", bufs=1) as cellp, \
             tc.tile_pool(name="wrk2", bufs=1) as wrk2, \
             tc.tile_pool(name="pfb", bufs=1) as pfb, \
             tc.tile_pool(name="pg", bufs=3, space="PSUM") as pg, \
             tc.tile_pool(name="pmix", bufs=3, space="PSUM") as pmix, \
             tc.tile_pool(name="pfc", bufs=1, space="PSUM") as pfc, \
             tc.tile_pool(name="dram", bufs=1, space="DRAM") as dram:

            bf16 = dt.bfloat16
            f32 = dt.float32

            # ---------- resident loads ----------
            def load(pool, src, shape, tag):
                t = pool.tile(shape, bf16, tag=tag)
                nc.sync.dma_start(t[:], src[:].rearrange(
                    "(k p) m -> p k m", p=128) if len(shape) == 3 else src[:])
                return t

            # weights stored [128, ktiles, width]
            w1aT_s = load(kw, w1aT, [128, KD, GS], "w1aT")
            w1hT_s = load(kw, w1hT, [128, KD, GS], "w1hT")
            w2aT_s = load(kw, w2aT, [128, KF, GS], "w2aT")
            w2bT_s = load(kw, w2bT, [128, KD, GS], "w2bT")
            w2hT_s = load(kw, w2hT, [128, KD, GS], "w2hT")
            wdT_s = load(kw, wdT, [128, KD, DS], "wdT")
            wfcT_s = load(kw, wfcT, [128, KD, VS], "wfcT")
            wacol_s = load(kw, wacol, [128, 1], "wacol")
            eye_s = load(kw, eye, [128, 128], "eye")
            actm_s = kw.tile([128, T], f32, tag="actm")
            nc.sync.dma_start(actm_s[:], actm[:])
            bg2_s = kw.tile([1, GS], bf16, tag="bg2")
            nc.sync.dma_start(bg2_s[:], bg2[:])
            bfc_s = kw.tile([1, VS], bf16, tag="bfc")
            nc.sync.dma_start(bfc_s[:], bfc[:])

            ones_s = kw.tile([1, CW], bf16, tag="ones")
            nc.vector.memset(ones_s[:], 1.0)

            # persistent state / gathered tensors
            att1T_s = kst.tile([128, NB], bf16, tag="att1T")      # [a, (n,b)]
            uc_s = kst.tile([128, 6, GS], bf16, tag="uc")         # U[t] rotating
            m_s = kst.tile([128, N, GS], bf16, tag="m_s")         # M_n  [b,(n,gs)]
            h1T_s = kst.tile([128, KD, 128], bf16, tag="h1T")     # gathered h1T
            h2T_s = kst.tile([128, KD, 128], bf16, tag="h2T")     # gathered h2T
            dch_s = kst.tile([128, N, 128], bf16, tag="dch")      # diag(alpha_n)
            c1_s = kst.tile([128, DS], f32, tag="c1")
            c2_s = kst.tile([128, DS], f32, tag="c2")
            nc.vector.memset(c1_s[:], 0.0)
            nc.vector.memset(c2_s[:], 0.0)

            # DRAM bounce buffers; collective outputs in Shared space
            ag1_in = dram.tile([128, 128], bf16, tag="ag1i")
            ag2_in = dram.tile([1, NB], bf16, tag="ag2i")
            ag4_in = dram.tile([128, 128], bf16, tag="ag4i")
            ag1_out = nc.dram_tensor("ag1o", [NC * 128, 128], bf16,
                                     addr_space="Shared").ap()
            ag2_out = nc.dram_tensor("ag2o", [1, NB], bf16,
                                     addr_space="Shared").ap()
            ag4_out = nc.dram_tensor("ag4o", [NC * 128, 128], bf16,
                                     addr_space="Shared").ap()

            # ---------- precompute: U1 (fmean + bias) ----------
            w1cT_s = pre.tile([128, KD, GS], bf16, tag="w1cT")
            nc.sync.dma_start(w1cT_s[:], w1cT[:].rearrange("(k p) m -> p k m", p=128))
            wfT_s = pre.tile([128, KF, DS], bf16, tag="wfT")
            nc.sync.dma_start(wfT_s[:], wfT[:].rearrange("(k p) m -> p k m", p=128))
            bg1_s = pre.tile([1, GS], bf16, tag="bg1")
            nc.sync.dma_start(bg1_s[:], bg1[:])
            batt_s = pre.tile([1, DS], bf16, tag="batt")
            nc.sync.dma_start(batt_s[:], batt[:])
            u1_sb = pre.tile([128, GS], f32, tag="u1")

            u1_ps = pg.tile([128, GS], f32, tag="pg")
            for k in range(KF):
                fm = ld.tile([128, 128], bf16, tag="fmch")
                nc.sync.dma_start(fm[:], fmeanT[k * 128:(k + 1) * 128, :])
                wb = ld.tile([128, GS], bf16, tag="wbch")
                nc.sync.dma_start(wb[:], w1bT[k * 128:(k + 1) * 128, :])
                nc.tensor.matmul(u1_ps[:], fm[:], wb[:],
                                 start=(k == 0), stop=False)
            nc.tensor.matmul(u1_ps[:], ones_s[0:1, 0:128], bg1_s[:],
                             start=False, stop=True)
            nc.vector.tensor_copy(u1_sb[:], u1_ps[:])

            # ---------- precompute: Uemb[t] (t=0..3 now, rest in-loop) ----
            def emit_uemb(t):
                et = ld.tile([128, KD, 128], bf16, tag="embt")
                nc.sync.dma_start(
                    et[:], embsT[t * E:(t + 1) * E, :].rearrange(
                        "(k p) m -> p k m", p=128))
                ue_ps = pg.tile([128, GS], f32, tag="pg")
                for k in range(KD):
                    nc.tensor.matmul(ue_ps[:], et[:, k, :], w1cT_s[:, k, :],
                                     start=(k == 0), stop=(k == KD - 1))
                nc.vector.tensor_tensor(uc_s[:, t % 6, :], ue_ps[:], u1_sb[:], OP.add)

            emit_uemb(0)

            # ---------- precompute: att1T ([a, (n, b)], A-sliced) ----------
            for cg in range(3):  # column groups of 1536 (3 psum chunks each)
                a1_pss = []
                for _cc in range(3):
                    a1c = pmix.tile([128, CW], f32, tag="pmix")
                    a1_pss.append(a1c)
                for k in range(KF):
                    fch = ld.tile([128, 3 * CW], bf16, tag="fch")
                    nc.scalar.dma_start(
                        fch[:], featsT[k * 128:(k + 1) * 128,
                                       cg * 3 * CW:(cg + 1) * 3 * CW])
                    for cc in range(3):
                        nc.tensor.matmul(
                            a1_pss[cc][:], wfT_s[:, k, :],
                            fch[:, cc * CW:(cc + 1) * CW],
                            start=(k == 0), stop=False)
                for cc in range(3):
                    c = cg * 3 + cc
                    nc.tensor.matmul(a1_pss[cc][:], batt_s[:],
                                     ones_s[0:1, 0:CW],
                                     start=False, stop=True)
                    nc.vector.tensor_copy(att1T_s[:, c * CW:(c + 1) * CW],
                                          a1_pss[cc][:])

            # ---------- precompute: M_n = feats_n @ W2a_slice.T ----------
            for n in range(N):
                ftile = ld.tile([128, KF, 128], bf16, tag="ftile")
                nc.sync.dma_start(
                    ftile[:], featsN[:, n * 128:(n + 1) * 128].rearrange(
                        "(k p) m -> p k m", p=128))
                m_ps = pg.tile([128, GS], f32, tag="pg")
                for k in range(KF):
                    nc.tensor.matmul(m_ps[:], ftile[:, k, :], w2aT_s[:, k, :],
                                     start=(k == 0), stop=(k == KF - 1))
                eng = nc.vector if n % 2 == 0 else nc.scalar
                if n % 2 == 0:
                    nc.vector.tensor_copy(m_s[:, n, :], m_ps[:])
                else:
                    nc.scalar.copy(m_s[:, n, :], m_ps[:])

            for t in range(1, 4):
                emit_uemb(t)

            # ---------- step loop ----------
            g1_ps = None
            g2_ps = None

            def open_gates(t):
                """g1 = U[t] (+ h1T(t-1) block), g2 = bias; emitted during
                the AG4(t-1) flight (h1T(t-1) is already gathered). The g1
                group is closed here for t==0 (no h2 block follows)."""
                g1 = pg.tile([128, GS], f32, tag="pg")
                nc.tensor.matmul(g1[:], eye_s[:], uc_s[:, t % 6, :],
                                 start=True, stop=(t == 0))
                if t > 0:
                    for k in range(KD):
                        nc.tensor.matmul(g1[:], h1T_s[:, k, :],
                                         w1hT_s[:, k, :], start=False,
                                         stop=False)
                g2 = pg.tile([128, GS], f32, tag="pg")
                nc.tensor.matmul(g2[:], ones_s[0:1, 0:128], bg2_s[:],
                                 start=True, stop=False)
                return g1, g2

            g1_ps, g2_ps = open_gates(0)

            for t in range(T):
                # --- post-AG4(t-1): h2 blocks of g1 and g2 ---
                if t > 0:
                    for k in range(KD):
                        nc.tensor.matmul(g1_ps[:], h2T_s[:, k, :],
                                         w1aT_s[:, k, :], start=False,
                                         stop=(k == KD - 1))
                    for k in range(KD):
                        nc.tensor.matmul(g2_ps[:], h2T_s[:, k, :],
                                         w2hT_s[:, k, :], start=False,
                                         stop=False)

                # --- cell 1 -> h1 bf16, transpose, AG1 ---
                h1_bf = _cell(nc, tc, cellp, wrk, g1_ps, c1_s, AF, OP)
                twarm = wrk.tile([1, 1], f32, tag="twarm")
                nc.scalar.activation(twarm[:], h1_bf[0:1, 0:1], AF.Exp)
                h1T_ps = pmix.tile([128, 128], bf16, tag="pmix")
                nc.tensor.transpose(h1T_ps[:], h1_bf[:], eye_s[:])
                h1T_loc = wrk.tile([128, 128], bf16, tag="hTloc")
                nc.vector.tensor_copy(h1T_loc[:], h1T_ps[:])
                nc.sync.dma_start(ag1_in[:], h1T_loc[:])
                nc.gpsimd.collective_compute(
                    "AllGather", AG, replica_groups=RG,
                    ins=[ag1_in.opt()], outs=[ag1_out.opt()])

                # --- AG1 gap: FC(t-1) both chunks + Uemb(t+4) ---
                if t > 0:
                    _emit_fc(nc, t - 1, 0, pfc, pfb, h2T_s, wfcT_s, ones_s,
                             bfc_s, actm_s, preds_o, KD, f32)
                    _emit_fc(nc, t - 1, 1, pfc, pfb, h2T_s, wfcT_s, ones_s,
                             bfc_s, actm_s, preds_o, KD, f32)
                if 4 + t < T:
                    emit_uemb(4 + t)
                nc.sync.dma_start(
                    h1T_s[:, 0:4, :],
                    ag1_out[0:512, :].rearrange("(k p) m -> p k m", p=128))
                nc.scalar.dma_start(
                    h1T_s[:, 4:8, :],
                    ag1_out[512:1024, :].rearrange("(k p) m -> p k m", p=128))

                # --- att2T = Wd_slice @ h1 ([a, b]) ---
                at2_ps = pmix.tile([128, 128], f32, tag="pmix")
                for k in range(KD):
                    nc.tensor.matmul(at2_ps[:], wdT_s[:, k, :], h1T_s[:, k, :],
                                     start=(k == 0), stop=(k == KD - 1))
                at2_bf = wrk.tile([128, 128], bf16, tag="at2")
                nc.vector.tensor_copy(at2_bf[:], at2_ps[:])

                # --- rt = relu(att1T + at2T) in [a, (b, n)]; e chunks ---
                rt = kst.tile([128, 128, N], bf16, tag="rt")
                for bq in range(4):
                    js = slice(bq * 32, (bq + 1) * 32)
                    nc.vector.tensor_tensor(
                        rt[:, js, :],
                        att1T_s[:, bq * 32 * N:(bq + 1) * 32 * N]
                        .rearrange("p (b n) -> p b n", n=N),
                        at2_bf[:, js].rearrange("p (j o) -> p j o", o=1)
                        .broadcast_to((128, 32, N)), OP.add)
                    nc.vector.tensor_scalar_max(
                        rt[:, js, :], rt[:, js, :], 0.0)
                rtf = rt[:].rearrange("p b n -> p (b n)")
                e_full = wrk2.tile([1, NB], bf16, tag="erow")
                for c in range(NCHUNK):
                    e_ps = pmix.tile([1, CW], f32, tag="pmix")
                    nc.tensor.matmul(e_ps[:], wacol_s[:],
                                     rtf[:, c * CW:(c + 1) * CW],
                                     start=True, stop=True)
                    if c % 2 == 0:
                        nc.scalar.copy(e_full[:, c * CW:(c + 1) * CW], e_ps[:])
                    else:
                        nc.vector.tensor_copy(
                            e_full[:, c * CW:(c + 1) * CW], e_ps[:])
                nc.sync.dma_start(ag2_in[:], e_full[:])

                # --- AR2: sum e partials across cores ---
                nc.gpsimd.collective_compute(
                    "AllReduce", OP.add, replica_groups=RG,
                    ins=[ag2_in.opt()], outs=[ag2_out.opt()])

                # --- AR2 gap: g2 h1-block ---
                for k in range(KD):
                    nc.tensor.matmul(g2_ps[:], h1T_s[:, k, :], w2bT_s[:, k, :],
                                     start=False, stop=False)

                # --- softmax (replicated, [b, n]) ---
                e_sb = wrk.tile([128, N], bf16, tag="esb")
                nc.sync.dma_start(e_sb[:], ag2_out[:].rearrange(
                    "o (b n) -> (o b) n", n=N))
                emax = wrk.tile([128, 1], f32, tag="emax")
                nc.vector.tensor_reduce(emax[:], e_sb[:], AX.X, OP.max,
                                        negate=True)
                expo = wrk.tile([128, N], f32, tag="expo")
                nc.scalar.activation(expo[:], e_sb[:], AF.Exp, bias=emax[:])
                twarm2 = wrk.tile([1, 1], f32, tag="twarm2")
                nc.scalar.activation(twarm2[:], expo[0:1, 0:1], AF.Sigmoid)
                esum = wrk.tile([128, 1], f32, tag="esum")
                nc.vector.tensor_reduce(esum[:], expo[:], AX.X, OP.add)
                erec = wrk.tile([128, 1], f32, tag="erec")
                nc.vector.reciprocal(erec[:], esum[:])
                alpha_bf = wrk.tile([128, N], bf16, tag="alpha")
                nc.vector.tensor_scalar_mul(alpha_bf[:], expo[:], erec[:])

                # --- diag(alpha_n) builds + g2 += sum_n diag_n @ M_n ---
                eye_b = eye_s[:].rearrange("p (o b) -> p o b", o=1)
                for nh in range(2):
                    ns = slice(nh * 18, (nh + 1) * 18)
                    nc.vector.tensor_tensor(
                        dch_s[:, ns, :],
                        eye_b.broadcast_to((128, 18, 128)),
                        alpha_bf[:, ns].rearrange("p (n o) -> p n o", o=1)
                        .broadcast_to((128, 18, 128)), OP.mult)
                    for n in range(nh * 18, (nh + 1) * 18):
                        nc.tensor.matmul(g2_ps[:], dch_s[:, n, :],
                                         m_s[:, n, :], start=False,
                                         stop=(n == N - 1))

                # --- cell 2 -> h2, transpose, AG4 ---
                h2_bf = _cell(nc, tc, cellp, wrk, g2_ps, c2_s, AF, OP)
                h2T_ps = pmix.tile([128, 128], bf16, tag="pmix")
                nc.tensor.transpose(h2T_ps[:], h2_bf[:], eye_s[:])
                h2T_loc = wrk.tile([128, 128], bf16, tag="hTloc")
                nc.vector.tensor_copy(h2T_loc[:], h2T_ps[:])
                nc.sync.dma_start(ag4_in[:], h2T_loc[:])
                nc.gpsimd.collective_compute(
                    "AllGather", AG, replica_groups=RG,
                    ins=[ag4_in.opt()], outs=[ag4_out.opt()])

                # --- AG4 gap: next step's g1 U/h1 blocks + g2 bias ---
                if t + 1 < T:
                    g1_ps, g2_ps = open_gates(t + 1)
                nc.sync.dma_start(
                    h2T_s[:, 0:4, :],
                    ag4_out[0:512, :].rearrange("(k p) m -> p k m", p=128))
                nc.scalar.dma_start(
                    h2T_s[:, 4:8, :],
                    ag4_out[512:1024, :].rearrange("(k p) m -> p k m", p=128))

            # final FC for last step
            _emit_fc(nc, T - 1, 0, pfc, pfb, h2T_s, wfcT_s, ones_s, bfc_s,
                     actm_s, preds_o, KD, f32)
            _emit_fc(nc, T - 1, 1, pfc, pfb, h2T_s, wfcT_s, ones_s, bfc_s,
                     actm_s, preds_o, KD, f32)

    nc.compile()
    return nc


def _cell(nc, tc, cellp, wrk, g_ps, c_s, AF, OP):
    """LSTM cell elementwise: gates [128, 512] f32 PSUM -> h bf16 [128,128].
    Updates c_s in place. Gate layout: [i, f, g, o] blocks of DS."""
    from concourse import mybir
    bf16 = mybir.dt.bfloat16
    f32 = mybir.dt.float32
    i_s = cellp.tile([128, 2 * DS], f32, tag="cif")
    nc.scalar.activation(i_s[:], g_ps[:, 0:2 * DS], AF.Sigmoid)
    t_g = cellp.tile([128, DS], f32, tag="cg")
    nc.scalar.activation(t_g[:], g_ps[:, 2 * DS:3 * DS], AF.Tanh)
    o_s = cellp.tile([128, DS], f32, tag="co")
    nc.scalar.activation(o_s[:], g_ps[:, 3 * DS:4 * DS], AF.Sigmoid)
    t1 = wrk.tile([128, DS], f32, tag="t1")
    nc.vector.tensor_tensor(t1[:], i_s[:, DS:2 * DS], c_s[:], OP.mult)
    t2 = wrk.tile([128, DS], f32, tag="t2")
    nc.vector.tensor_tensor(t2[:], i_s[:, 0:DS], t_g[:], OP.mult)
    nc.vector.tensor_tensor(c_s[:], t1[:], t2[:], OP.add)
    tc2 = wrk.tile([128, DS], f32, tag="tc2")
    nc.scalar.activation(tc2[:], c_s[:], AF.Tanh)
    h_bf = wrk.tile([128, DS], bf16, tag="hbf")
    nc.vector.tensor_tensor(h_bf[:], o_s[:], tc2[:], OP.mult)
    return h_bf


def _emit_fc(nc, t, half, pfc, pfb, h2T_s, wfcT_s, ones_s, bfc_s, actm_s,
             preds_o, KD, f32):
    """logits chunk for step t: [128, VH] = h2(t) @ WfcT[:, lo:hi] + bfc,
    masked by active."""
    lo0, hi0 = half * VH, (half + 1) * VH
    fc_ps = pfc.tile([128, VH], f32, tag="pfc")
    p_sb = pfb.tile([128, VH], f32, tag="psb")
    for lo in range(lo0, hi0, 512):
        hi = min(lo + 512, hi0)
        for k in range(KD):
            nc.tensor.matmul(fc_ps[:, lo - lo0:hi - lo0], h2T_s[:, k, :],
                             wfcT_s[:, k, lo:hi], start=(k == 0), stop=False)
        nc.tensor.matmul(fc_ps[:, lo - lo0:hi - lo0], ones_s[0:1, 0:128],
                         bfc_s[:, lo:hi], start=False, stop=True)
    nc.vector.tensor_scalar_mul(p_sb[:], fc_ps[:], actm_s[:, t:t + 1])
    nc.sync.dma_start(preds_o[t * B:(t + 1) * B, lo0:hi0], p_sb[:])


def _host_prep(inputs):
    """Sort, gather, transpose, cast, slice per core."""
    f32 = np.float32
    lengths = np.asarray(inputs["caption_lengths"])[:, 0]
    sort_ind = np.argsort(-lengths, kind="stable")
    feats = np.asarray(inputs["image_features"], f32)[sort_ind]        # [B,N,F]
    caps = np.asarray(inputs["encoded_captions"])[sort_ind]            # [B,L]
    dec_len = lengths[sort_ind] - 1
    emb = np.asarray(inputs["emb"], f32)
    embs = emb[caps[:, :T]]                                            # [B,T,E]
    fmean = feats.mean(axis=1)                                         # [B,F]

    # featsT b-major [f, (b, n)]; featsN n-major [f, (n, b)]
    featsT = np.ascontiguousarray(feats.transpose(2, 0, 1)).reshape(F, NB)
    featsN = np.ascontiguousarray(feats.transpose(2, 1, 0)).reshape(F, NB)
    embsT = np.ascontiguousarray(embs.transpose(1, 2, 0)).reshape(T * E, B)
    fmeanT = np.ascontiguousarray(fmean.T)                             # [F,B]
    actm = (np.arange(T)[None, :] < dec_len[:, None]).astype(f32)      # [B,T]
    eye = np.eye(128, dtype=BF)

    W1 = np.asarray(inputs["W1_ih"], f32); W1h = np.asarray(inputs["W1_hh"], f32)
    W2 = np.asarray(inputs["W2_ih"], f32); W2h = np.asarray(inputs["W2_hh"], f32)
    Wf = np.asarray(inputs["Wf"], f32); Wd = np.asarray(inputs["Wd"], f32)
    Wa = np.asarray(inputs["Wa"], f32); Wfc = np.asarray(inputs["Wfc"], f32)
    b1 = np.asarray(inputs["b1_ih"], f32) + np.asarray(inputs["b1_hh"], f32)
    b2 = np.asarray(inputs["b2_ih"], f32) + np.asarray(inputs["b2_hh"], f32)
    bfv = np.asarray(inputs["bf"], f32) + np.asarray(inputs["bd"], f32)
    bfc = np.asarray(inputs["bfc"], f32)

    shared = {
        "featsT": featsT.astype(BF), "featsN": featsN.astype(BF),
        "embsT": embsT.astype(BF),
        "fmeanT": fmeanT.astype(BF), "eye": eye, "actm": actm,
    }
    tp = lambda x: np.ascontiguousarray(x.T).astype(BF)
    in_maps = []
    for i in range(NC):
        rows = np.concatenate([np.arange(q * D + i * DS, q * D + (i + 1) * DS)
                               for q in range(4)])
        asl = slice(i * DS, (i + 1) * DS)
        m = dict(shared)
        m["w1aT"] = tp(W1[rows, 0:D])
        m["w1bT"] = tp(W1[rows, D:D + F])
        m["w1cT"] = tp(W1[rows, D + F:])
        m["w1hT"] = tp(W1h[rows])
        m["w2aT"] = tp(W2[rows, 0:F])
        m["w2bT"] = tp(W2[rows, F:])
        m["w2hT"] = tp(W2h[rows])
        m["wdT"] = tp(Wd[asl])
        m["wfT"] = tp(Wf[asl])
        m["wacol"] = np.ascontiguousarray(Wa[0, asl])[:, None].astype(BF)
        m["wfcT"] = tp(Wfc[i * VS:(i + 1) * VS])
        m["bg1"] = b1[rows][None, :].astype(BF)
        m["bg2"] = b2[rows][None, :].astype(BF)
        m["batt"] = bfv[asl][None, :].astype(BF)
        m["bfc"] = bfc[i * VS:(i + 1) * VS][None, :].astype(BF)
        in_maps.append(m)
    return in_maps


def kernel(**inputs):
    global _PROG
    from concourse.bass_utils import run_bass_kernel_spmd
    if _PROG is None:
        _PROG = _build()
    in_maps = _host_prep(inputs)
    res = run_bass_kernel_spmd(
        _PROG, in_maps, core_ids=list(range(NC)),
        trace=os.environ.get("KERNEL_TRACE") == "1")
    if res.exec_time_ns is not None:
        kernel.last_exec_time_ns = res.exec_time_ns
    preds = np.concatenate(
        [res.results[i]["preds"].reshape(T, B, VS) for i in range(NC)], axis=2)
    return np.ascontiguousarray(preds.transpose(1, 0, 2))


# revision 11
# speedup vs baseline: 1.3118x; 1.0573x over previous
"""Trainium2 Bass kernel for nn_DecoderWithAttention (Show-Attend-Tell decoder).

v2 strategy (8 NeuronCores, tensor-parallel, 3 collectives/step):
 - Gate/hidden dims of both LSTMs, attention dim A, and vocab V sharded 8
   ways; batch B=128 whole on every core as the partition dim.
 - awe is never materialized: since g2_awe = awe @ W2a.T with
   awe = sum_n alpha_n * feats_n, we precompute M_n = feats[:,n,:] @ W2a.T
   (per-core gate slice) once, and per step accumulate
   g2 += sum_n diag(alpha[:,n]) @ M_n directly in PSUM (36 matmuls).
   This removes the aweT AllGather (AG3), its transposes, and featsaw.
 - Per step: AG1 (h1T 32KB), AR2 (e-partials 9KB), AG4 (h2T 32KB); all
   collective outputs in Shared DRAM address space.
 - U[t] (emb/fmean gate constant) and biases are injected into PSUM via
   identity/ones matmuls; LSTM cells read gates straight from PSUM.
 - FC (logits, V-sharded) is split in two chunks filling the AG1 and AR2
   gaps; Uemb prefetch also fills the AR2 gap; next-step g1 h1-block +
   U-load + g2 bias fill the AG4 gap.
 - att1T/featsT/e use b-major layout [a, (b, n)] so the e AllReduce output
   loads as [b, n] with contiguous per-partition DMA segments.

Host side: stable argsort by length (reference returns the sorted batch
order), embedding gather, transposes/casts to bf16, weight slicing per core.
"""
import sys, os
sys.path.insert(0, "/opt/trn_rl_repo")

import numpy as np
import ml_dtypes

BF = ml_dtypes.bfloat16

# problem dims (hardcoded per the task contract)
B, N, F, A, E, D, V, L = 128, 36, 2048, 1024, 1024, 1024, 10000, 20
T = L - 1                       # 19 decode steps
NC = 8                          # cores
DS = D // NC                    # 128   hidden slice
GS = 4 * DS                     # 512   gate slice (i,f,g,o blocks of DS)
VS = V // NC                    # 1250  vocab slice
VH = 625                        # FC chunk width (2 chunks)
KD = D // 128                   # 8     k-tiles over D
KF = F // 128                   # 16    k-tiles over F
NB = N * B                      # 4608  (n, b) flattened
NCHUNK = 9                      # e chunks over (n, b)
CW = NB // NCHUNK               # 512   chunk width (4 n's)

_PROG = None  # cached build


def _build():
    from concourse import bass, tile, mybir, bacc

    dt = mybir.dt
    nc = bacc.Bacc("TRN2", target_bir_lowering=False, debug=False,
                   num_devices=NC)

    def din(name, shape, d=dt.bfloat16):
        return nc.dram_tensor(name, shape, d, kind="ExternalInput").ap()

    # ---- inputs (per-core unless noted shared) ----
    featsT = din("featsT", [F, NB])            # shared  [f, (b, n)]
    featsN = din("featsN", [F, NB])            # shared  [f, (n, b)]
    embsT = din("embsT", [T * E, B])           # shared  [(t, e), b]
    fmeanT = din("fmeanT", [F, B])             # shared  [f, b]
    eye = din("eye", [128, 128])               # shared  identity
    actm = din("actm", [B, T], dt.float32)     # shared  active mask
    w1aT = din("w1aT", [D, GS])                # W1_ih[rows, :D].T      (h2 block)
    w1hT = din("w1hT", [D, GS])                # W1_hh[rows].T
    w1bT = din("w1bT", [F, GS])                # W1_ih[rows, D:D+F].T   (fmean)
    w1cT = din("w1cT", [E, GS])                # W1_ih[rows, D+F:].T    (emb)
    w2aT = din("w2aT", [F, GS])                # W2_ih[rows, :F].T      (awe)
    w2bT = din("w2bT", [D, GS])                # W2_ih[rows, F:].T      (h1)
    w2hT = din("w2hT", [D, GS])                # W2_hh[rows].T
    wdT = din("wdT", [D, DS])                  # Wd[a_slice].T
    wfT = din("wfT", [F, DS])                  # Wf[a_slice].T
    wacol = din("wacol", [DS, 1])              # Wa[0, a_slice] column
    wfcT = din("wfcT", [D, VS])                # Wfc[v_slice].T
    bg1 = din("bg1", [1, GS])                  # (b1_ih+b1_hh)[rows]
    bg2 = din("bg2", [1, GS])                  # (b2_ih+b2_hh)[rows]
    batt = din("batt", [1, DS])                # (bf+bd)[a_slice]
    bfc = din("bfc", [1, VS])                  # bfc[v_slice]

    preds_o = nc.dram_tensor("preds", [T * B, VS], dt.float32,
                             kind="ExternalOutput").ap()

    AG = mybir.AluOpType.bypass
    AF = mybir.ActivationFunctionType
    OP = mybir.AluOpType
    AX = mybir.AxisListType
    RG = [list(range(NC))]

    with tile.TileContext(nc) as tc:
        with tc.tile_pool(name="kw", bufs=1) as kw, \
             tc.tile_pool(name="kst", bufs=1) as kst, \
             tc.tile_pool(name="pre", bufs=1) as pre, \
             tc.tile_pool(name="ld", bufs=2) as ld, \
             tc.tile_pool(name="wrk", bufs=2) as wrk, \
             tc.tile_pool(name="cell", bufs=1) as cellp, \
             tc.tile_pool(name="wrk2", bufs=1) as wrk2, \
             tc.tile_pool(name="pfb", bufs=1) as pfb, \
             tc.tile_pool(name="pg", bufs=3, space="PSUM") as pg, \
             tc.tile_pool(name="pmix", bufs=3, space="PSUM") as pmix, \
             tc.tile_pool(name="pfc", bufs=1, space="PSUM") as pfc, \
             tc.tile_pool(name="dram", bufs=1, space="DRAM") as dram:

            bf16 = dt.bfloat16
            f32 = dt.float32

            # ---------- resident loads ----------
            def load(pool, src, shape, tag):
                t = pool.tile(shape, bf16, tag=tag)
                nc.sync.dma_start(t[:], src[:].rearrange(
                    "(k p) m -> p k m", p=128) if len(shape) == 3 else src[:])
                return t

            # weights stored [128, ktiles, width]
            w1aT_s = load(kw, w1aT, [128, KD, GS], "w1aT")
            w1hT_s = load(kw, w1hT, [128, KD, GS], "w1hT")
            w2aT_s = load(kw, w2aT, [128, KF, GS], "w2aT")
            w2bT_s = load(kw, w2bT, [128, KD, GS], "w2bT")
            w2hT_s = load(kw, w2hT, [128, KD, GS], "w2hT")
            wdT_s = load(kw, wdT, [128, KD, DS], "wdT")
            wfcT_s = load(kw, wfcT, [128, KD, VS], "wfcT")
            wacol_s = load(kw, wacol, [128, 1], "wacol")
            eye_s = load(kw, eye, [128, 128], "eye")
            actm_s = kw.tile([128, T], f32, tag="actm")
            nc.sync.dma_start(actm_s[:], actm[:])
            bg2_s = kw.tile([1, GS], bf16, tag="bg2")
            nc.sync.dma_start(bg2_s[:], bg2[:])
            bfc_s = kw.tile([1, VS], bf16, tag="bfc")
            nc.sync.dma_start(bfc_s[:], bfc[:])

            ones_s = kw.tile([1, CW], bf16, tag="ones")
            nc.vector.memset(ones_s[:], 1.0)

            # persistent state / gathered tensors
            att1T_s = kst.tile([128, NB], bf16, tag="att1T")      # [a, (n,b)]
            uc_s = kst.tile([128, 6, GS], bf16, tag="uc")         # U[t] rotating
            m_s = kst.tile([128, N, GS], bf16, tag="m_s")         # M_n  [b,(n,gs)]
            h1T_s = kst.tile([128, KD, 128], bf16, tag="h1T")     # gathered h1T
            h2T_s = kst.tile([128, KD, 128], bf16, tag="h2T")     # gathered h2T
            dch_s = kst.tile([128, N, 128], bf16, tag="dch")      # diag(alpha_n)
            c1_s = kst.tile([128, DS], f32, tag="c1")
            c2_s = kst.tile([128, DS], f32, tag="c2")
            nc.vector.memset(c1_s[:], 0.0)
            nc.vector.memset(c2_s[:], 0.0)

            # DRAM bounce buffers; collective outputs in Shared space
            ag1_in = dram.tile([128, 128], bf16, tag="ag1i")
            ag2_in = dram.tile([1, NB], bf16, tag="ag2i")
            ag4_in = dram.tile([128, 128], bf16, tag="ag4i")
            ag1_out = nc.dram_tensor("ag1o", [NC * 128, 128], bf16,
                                     addr_space="Shared").ap()
            ag2_out = nc.dram_tensor("ag2o", [1, NB], bf16,
                                     addr_space="Shared").ap()
            ag4_out = nc.dram_tensor("ag4o", [NC * 128, 128], bf16,
                                     addr_space="Shared").ap()

            # ---------- precompute: U1 (fmean + bias) ----------
            w1cT_s = pre.tile([128, KD, GS], bf16, tag="w1cT")
            nc.sync.dma_start(w1cT_s[:], w1cT[:].rearrange("(k p) m -> p k m", p=128))
            wfT_s = pre.tile([128, KF, DS], bf16, tag="wfT")
            nc.sync.dma_start(wfT_s[:], wfT[:].rearrange("(k p) m -> p k m", p=128))
            bg1_s = pre.tile([1, GS], bf16, tag="bg1")
            nc.sync.dma_start(bg1_s[:], bg1[:])
            batt_s = pre.tile([1, DS], bf16, tag="batt")
            nc.sync.dma_start(batt_s[:], batt[:])
            u1_sb = pre.tile([128, GS], f32, tag="u1")

            u1_ps = pg.tile([128, GS], f32, tag="pg")
            for k in range(KF):
                fm = ld.tile([128, 128], bf16, tag="fmch")
                nc.sync.dma_start(fm[:], fmeanT[k * 128:(k + 1) * 128, :])
                wb = ld.tile([128, GS], bf16, tag="wbch")
                nc.sync.dma_start(wb[:], w1bT[k * 128:(k + 1) * 128, :])
                nc.tensor.matmul(u1_ps[:], fm[:], wb[:],
                                 start=(k == 0), stop=False)
            nc.tensor.matmul(u1_ps[:], ones_s[0:1, 0:128], bg1_s[:],
                             start=False, stop=True)
            nc.vector.tensor_copy(u1_sb[:], u1_ps[:])

            # ---------- precompute: Uemb[t] (t=0..3 now, rest in-loop) ----
            def emit_uemb(t):
                et = ld.tile([128, KD, 128], bf16, tag="embt")
                nc.sync.dma_start(
                    et[:], embsT[t * E:(t + 1) * E, :].rearrange(
                        "(k p) m -> p k m", p=128))
                ue_ps = pg.tile([128, GS], f32, tag="pg")
                for k in range(KD):
                    nc.tensor.matmul(ue_ps[:], et[:, k, :], w1cT_s[:, k, :],
                                     start=(k == 0), stop=(k == KD - 1))
                nc.vector.tensor_tensor(uc_s[:, t % 6, :], ue_ps[:], u1_sb[:], OP.add)

            emit_uemb(0)

            # ---------- precompute: att1T ([a, (n, b)], A-sliced) ----------
            for cg in range(3):  # column groups of 1536 (3 psum chunks each)
                a1_pss = []
                for _cc in range(3):
                    a1c = pmix.tile([128, CW], f32, tag="pmix")
                    a1_pss.append(a1c)
                for k in range(KF):
                    fch = ld.tile([128, 3 * CW], bf16, tag="fch")
                    nc.scalar.dma_start(
                        fch[:], featsT[k * 128:(k + 1) * 128,
                                       cg * 3 * CW:(cg + 1) * 3 * CW])
                    for cc in range(3):
                        nc.tensor.matmul(
                            a1_pss[cc][:], wfT_s[:, k, :],
                            fch[:, cc * CW:(cc + 1) * CW],
                            start=(k == 0), stop=False)
                for cc in range(3):
                    c = cg * 3 + cc
                    nc.tensor.matmul(a1_pss[cc][:], batt_s[:],
                                     ones_s[0:1, 0:CW],
                                     start=False, stop=True)
                    nc.vector.tensor_copy(att1T_s[:, c * CW:(c + 1) * CW],
                                          a1_pss[cc][:])

            # ---------- precompute: M_n = feats_n @ W2a_slice.T ----------
            for n in range(N):
                ftile = ld.tile([128, KF, 128], bf16, tag="ftile")
                nc.sync.dma_start(
                    ftile[:], featsN[:, n * 128:(n + 1) * 128].rearrange(
                        "(k p) m -> p k m", p=128))
                m_ps = pg.tile([128, GS], f32, tag="pg")
                for k in range(KF):
                    nc.tensor.matmul(m_ps[:], ftile[:, k, :], w2aT_s[:, k, :],
                                     start=(k == 0), stop=(k == KF - 1))
                eng = nc.vector if n % 2 == 0 else nc.scalar
                if n % 2 == 0:
                    nc.vector.tensor_copy(m_s[:, n, :], m_ps[:])
                else:
                    nc.scalar.copy(m_s[:, n, :], m_ps[:])

            for t in range(1, 4):
                emit_uemb(t)

            # ---------- step loop ----------
            g1_ps = None
            g2_ps = None

            def open_gates(t):
                """g1 = U[t] (+ h1T(t-1) block), g2 = bias; emitted during
                the AG4(t-1) flight (h1T(t-1) is already gathered). The g1
                group is closed here for t==0 (no h2 block follows)."""
                g1 = pg.tile([128, GS], f32, tag="pg")
                nc.tensor.matmul(g1[:], eye_s[:], uc_s[:, t % 6, :],
                                 start=True, stop=(t == 0))
                if t > 0:
                    for k in range(KD):
                        nc.tensor.matmul(g1[:], h1T_s[:, k, :],
                                         w1hT_s[:, k, :], start=False,
                                         stop=False)
                g2 = pg.tile([128, GS], f32, tag="pg")
                nc.tensor.matmul(g2[:], ones_s[0:1, 0:128], bg2_s[:],
                                 start=True, stop=False)
                return g1, g2

            g1_ps, g2_ps = open_gates(0)

            for t in range(T):
                # --- post-AG4(t-1): h2 blocks of g1 and g2 ---
                if t > 0:
                    for k in range(KD):
                        nc.tensor.matmul(g1_ps[:], h2T_s[:, k, :],
                                         w1aT_s[:, k, :], start=False,
                                         stop=(k == KD - 1))
                    for k in range(KD):
                        nc.tensor.matmul(g2_ps[:], h2T_s[:, k, :],
                                         w2hT_s[:, k, :], start=False,
                                         stop=False)

                # --- cell 1 -> h1 bf16, transpose, AG1 ---
                h1_bf = _cell(nc, tc, cellp, wrk, g1_ps, c1_s, AF, OP)
                twarm = wrk.tile([1, 1], f32, tag="twarm")
                nc.scalar.activation(twarm[:], h1_bf[0:1, 0:1], AF.Exp)
                h1T_ps = pmix.tile([128, 128], bf16, tag="pmix")
                nc.tensor.transpose(h1T_ps[:], h1_bf[:], eye_s[:])
                h1T_loc = wrk.tile([128, 128], bf16, tag="hTloc")
                nc.vector.tensor_copy(h1T_loc[:], h1T_ps[:])
                nc.sync.dma_start(ag1_in[:], h1T_loc[:])
                nc.gpsimd.collective_compute(
                    "AllGather", AG, replica_groups=RG,
                    ins=[ag1_in.opt()], outs=[ag1_out.opt()])

                # --- AG1 gap: FC(t-1) both chunks + Uemb(t+4) ---
                if t > 0:
                    _emit_fc(nc, t - 1, 0, pfc, pfb, h2T_s, wfcT_s, ones_s,
                             bfc_s, actm_s, preds_o, KD, f32)
                    _emit_fc(nc, t - 1, 1, pfc, pfb, h2T_s, wfcT_s, ones_s,
                             bfc_s, actm_s, preds_o, KD, f32)
                if 4 + t < T:
                    emit_uemb(4 + t)
                for kq in range(4):
                    eng = nc.sync if kq % 2 == 0 else nc.scalar
                    eng.dma_start(
                        h1T_s[:, 2 * kq:2 * kq + 2, :],
                        ag1_out[256 * kq:256 * (kq + 1), :].rearrange(
                            "(k p) m -> p k m", p=128))

                # --- att2T = Wd_slice @ h1 ([a, b]) ---
                at2_ps = pmix.tile([128, 128], f32, tag="pmix")
                for k in range(KD):
                    nc.tensor.matmul(at2_ps[:], wdT_s[:, k, :], h1T_s[:, k, :],
                                     start=(k == 0), stop=(k == KD - 1))
                at2_bf = wrk.tile([128, 128], bf16, tag="at2")
                nc.vector.tensor_copy(at2_bf[:], at2_ps[:])

                # --- rt = relu(att1T + at2T) in [a, (b, n)]; e chunks ---
                rt = kst.tile([128, 128, N], bf16, tag="rt")
                for bq in range(4):
                    js = slice(bq * 32, (bq + 1) * 32)
                    nc.vector.tensor_tensor(
                        rt[:, js, :],
                        att1T_s[:, bq * 32 * N:(bq + 1) * 32 * N]
                        .rearrange("p (b n) -> p b n", n=N),
                        at2_bf[:, js].rearrange("p (j o) -> p j o", o=1)
                        .broadcast_to((128, 32, N)), OP.add)
                    nc.vector.tensor_scalar_max(
                        rt[:, js, :], rt[:, js, :], 0.0)
                rtf = rt[:].rearrange("p b n -> p (b n)")
                e_full = wrk2.tile([1, NB], bf16, tag="erow")
                for c in range(NCHUNK):
                    e_ps = pmix.tile([1, CW], f32, tag="pmix")
                    nc.tensor.matmul(e_ps[:], wacol_s[:],
                                     rtf[:, c * CW:(c + 1) * CW],
                                     start=True, stop=True)
                    if c % 2 == 0:
                        nc.scalar.copy(e_full[:, c * CW:(c + 1) * CW], e_ps[:])
                    else:
                        nc.vector.tensor_copy(
                            e_full[:, c * CW:(c + 1) * CW], e_ps[:])
                nc.sync.dma_start(ag2_in[:], e_full[:])

                # --- AR2: sum e partials across cores ---
                nc.gpsimd.collective_compute(
                    "AllReduce", OP.add, replica_groups=RG,
                    ins=[ag2_in.opt()], outs=[ag2_out.opt()])

                # --- AR2 gap: g2 h1-block ---
                for k in range(KD):
                    nc.tensor.matmul(g2_ps[:], h1T_s[:, k, :], w2bT_s[:, k, :],
                                     start=False, stop=False)

                # --- softmax (replicated, [b, n]) ---
                e_sb = wrk.tile([128, N], bf16, tag="esb")
                nc.sync.dma_start(e_sb[:], ag2_out[:].rearrange(
                    "o (b n) -> (o b) n", n=N))
                emax = wrk.tile([128, 1], f32, tag="emax")
                nc.vector.tensor_reduce(emax[:], e_sb[:], AX.X, OP.max,
                                        negate=True)
                expo = wrk.tile([128, N], f32, tag="expo")
                nc.scalar.activation(expo[:], e_sb[:], AF.Exp, bias=emax[:])
                twarm2 = wrk.tile([1, 1], f32, tag="twarm2")
                nc.scalar.activation(twarm2[:], expo[0:1, 0:1], AF.Sigmoid)
                esum = wrk.tile([128, 1], f32, tag="esum")
                nc.vector.tensor_reduce(esum[:], expo[:], AX.X, OP.add)
                erec = wrk.tile([128, 1], f32, tag="erec")
                nc.vector.reciprocal(erec[:], esum[:])
                alpha_bf = wrk.tile([128, N], bf16, tag="alpha")
                nc.vector.tensor_scalar_mul(alpha_bf[:], expo[:], erec[:])

                # --- diag(alpha_n) builds + g2 += sum_n diag_n @ M_n ---
                eye_b = eye_s[:].rearrange("p (o b) -> p o b", o=1)
                for nh in range(4):
                    ns = slice(nh * 9, (nh + 1) * 9)
                    nc.vector.tensor_tensor(
                        dch_s[:, ns, :],
                        eye_b.broadcast_to((128, 9, 128)),
                        alpha_bf[:, ns].rearrange("p (n o) -> p n o", o=1)
                        .broadcast_to((128, 9, 128)), OP.mult)
                    for n in range(nh * 9, (nh + 1) * 9):
                        nc.tensor.matmul(g2_ps[:], dch_s[:, n, :],
                                         m_s[:, n, :], start=False,
                                         stop=(n == N - 1))

                # --- cell 2 -> h2, transpose, AG4 ---
                h2_bf = _cell(nc, tc, cellp, wrk, g2_ps, c2_s, AF, OP)
                h2T_ps = pmix.tile([128, 128], bf16, tag="pmix")
                nc.tensor.transpose(h2T_ps[:], h2_bf[:], eye_s[:])
                h2T_loc = wrk.tile([128, 128], bf16, tag="hTloc")
                nc.vector.tensor_copy(h2T_loc[:], h2T_ps[:])
                nc.sync.dma_start(ag4_in[:], h2T_loc[:])
                nc.gpsimd.collective_compute(
                    "AllGather", AG, replica_groups=RG,
                    ins=[ag4_in.opt()], outs=[ag4_out.opt()])

                # --- AG4 gap: next step's g1 U/h1 blocks + g2 bias ---
                if t + 1 < T:
                    g1_ps, g2_ps = open_gates(t + 1)
                for kq in range(4):
                    eng = nc.sync if kq % 2 == 0 else nc.scalar
                    eng.dma_start(
                        h2T_s[:, 2 * kq:2 * kq + 2, :],
                        ag4_out[256 * kq:256 * (kq + 1), :].rearrange(
                            "(k p) m -> p k m", p=128))

            # final FC for last step
            _emit_fc(nc, T - 1, 0, pfc, pfb, h2T_s, wfcT_s, ones_s, bfc_s,
                     actm_s, preds_o, KD, f32)
            _emit_fc(nc, T - 1, 1, pfc, pfb, h2T_s, wfcT_s, ones_s, bfc_s,
                     actm_s, preds_o, KD, f32)

    nc.compile()
    return nc


def _cell(nc, tc, cellp, wrk, g_ps, c_s, AF, OP):
    """LSTM cell elementwise: gates [128, 512] f32 PSUM -> h bf16 [128,128].
    Updates c_s in place. Gate layout: [i, f, g, o] blocks of DS."""
    from concourse import mybir
    bf16 = mybir.dt.bfloat16
    f32 = mybir.dt.float32
    i_s = cellp.tile([128, 2 * DS], f32, tag="cif")
    nc.scalar.activation(i_s[:], g_ps[:, 0:2 * DS], AF.Sigmoid)
    t_g = cellp.tile([128, DS], f32, tag="cg")
    nc.scalar.activation(t_g[:], g_ps[:, 2 * DS:3 * DS], AF.Tanh)
    o_s = cellp.tile([128, DS], f32, tag="co")
    nc.scalar.activation(o_s[:], g_ps[:, 3 * DS:4 * DS], AF.Sigmoid)
    t1 = wrk.tile([128, DS], f32, tag="t1")
    nc.vector.tensor_tensor(t1[:], i_s[:, DS:2 * DS], c_s[:], OP.mult)
    t2 = wrk.tile([128, DS], f32, tag="t2")
    nc.vector.tensor_tensor(t2[:], i_s[:, 0:DS], t_g[:], OP.mult)
    nc.vector.tensor_tensor(c_s[:], t1[:], t2[:], OP.add)
    tc2 = wrk.tile([128, DS], f32, tag="tc2")
    nc.scalar.activation(tc2[:], c_s[:], AF.Tanh)
    h_bf = wrk.tile([128, DS], bf16, tag="hbf")
    nc.vector.tensor_tensor(h_bf[:], o_s[:], tc2[:], OP.mult)
    return h_bf


def _emit_fc(nc, t, half, pfc, pfb, h2T_s, wfcT_s, ones_s, bfc_s, actm_s,
             preds_o, KD, f32):
    """logits chunk for step t: [128, VH] = h2(t) @ WfcT[:, lo:hi] + bfc,
    masked by active."""
    lo0, hi0 = half * VH, (half + 1) * VH
    fc_ps = pfc.tile([128, VH], f32, tag="pfc")
    p_sb = pfb.tile([128, VH], f32, tag="psb")
    for lo in range(lo0, hi0, 512):
        hi = min(lo + 512, hi0)
        for k in range(KD):
            nc.tensor.matmul(fc_ps[:, lo - lo0:hi - lo0], h2T_s[:, k, :],
                             wfcT_s[:, k, lo:hi], start=(k == 0), stop=False)
        nc.tensor.matmul(fc_ps[:, lo - lo0:hi - lo0], ones_s[0:1, 0:128],
                         bfc_s[:, lo:hi], start=False, stop=True)
    nc.vector.tensor_scalar_mul(p_sb[:], fc_ps[:], actm_s[:, t:t + 1])
    nc.scalar.dma_start(preds_o[t * B:(t + 1) * B, lo0:hi0], p_sb[:])


def _host_prep(inputs):
    """Sort, gather, transpose, cast, slice per core."""
    f32 = np.float32
    lengths = np.asarray(inputs["caption_lengths"])[:, 0]
    sort_ind = np.argsort(-lengths, kind="stable")
    feats = np.asarray(inputs["image_features"], f32)[sort_ind]        # [B,N,F]
    caps = np.asarray(inputs["encoded_captions"])[sort_ind]            # [B,L]
    dec_len = lengths[sort_ind] - 1
    emb = np.asarray(inputs["emb"], f32)
    embs = emb[caps[:, :T]]                                            # [B,T,E]
    fmean = feats.mean(axis=1)                                         # [B,F]

    # featsT b-major [f, (b, n)]; featsN n-major [f, (n, b)]
    featsT = np.ascontiguousarray(feats.transpose(2, 0, 1)).reshape(F, NB)
    featsN = np.ascontiguousarray(feats.transpose(2, 1, 0)).reshape(F, NB)
    embsT = np.ascontiguousarray(embs.transpose(1, 2, 0)).reshape(T * E, B)
    fmeanT = np.ascontiguousarray(fmean.T)                             # [F,B]
    actm = (np.arange(T)[None, :] < dec_len[:, None]).astype(f32)      # [B,T]
    eye = np.eye(128, dtype=BF)

    W1 = np.asarray(inputs["W1_ih"], f32); W1h = np.asarray(inputs["W1_hh"], f32)
    W2 = np.asarray(inputs["W2_ih"], f32); W2h = np.asarray(inputs["W2_hh"], f32)
    Wf = np.asarray(inputs["Wf"], f32); Wd = np.asarray(inputs["Wd"], f32)
    Wa = np.asarray(inputs["Wa"], f32); Wfc = np.asarray(inputs["Wfc"], f32)
    b1 = np.asarray(inputs["b1_ih"], f32) + np.asarray(inputs["b1_hh"], f32)
    b2 = np.asarray(inputs["b2_ih"], f32) + np.asarray(inputs["b2_hh"], f32)
    bfv = np.asarray(inputs["bf"], f32) + np.asarray(inputs["bd"], f32)
    bfc = np.asarray(inputs["bfc"], f32)

    shared = {
        "featsT": featsT.astype(BF), "featsN": featsN.astype(BF),
        "embsT": embsT.astype(BF),
        "fmeanT": fmeanT.astype(BF), "eye": eye, "actm": actm,
    }
    tp = lambda x: np.ascontiguousarray(x.T).astype(BF)
    in_maps = []
    for i in range(NC):
        rows = np.concatenate([np.arange(q * D + i * DS, q * D + (i + 1) * DS)
                               for q in range(4)])
        asl = slice(i * DS, (i + 1) * DS)
        m = dict(shared)
        m["w1aT"] = tp(W1[rows, 0:D])
        m["w1bT"] = tp(W1[rows, D:D + F])
        m["w1cT"] = tp(W1[rows, D + F:])
        m["w1hT"] = tp(W1h[rows])
        m["w2aT"] = tp(W2[rows, 0:F])
        m["w2bT"] = tp(W2[rows, F:])
        m["w2hT"] = tp(W2h[rows])
        m["wdT"] = tp(Wd[asl])
        m["wfT"] = tp(Wf[asl])
        m["wacol"] = np.ascontiguousarray(Wa[0, asl])[:, None].astype(BF)
        m["wfcT"] = tp(Wfc[i * VS:(i + 1) * VS])
        m["bg1"] = b1[rows][None, :].astype(BF)
        m["bg2"] = b2[rows][None, :].astype(BF)
        m["batt"] = bfv[asl][None, :].astype(BF)
        m["bfc"] = bfc[i * VS:(i + 1) * VS][None, :].astype(BF)
        in_maps.append(m)
    return in_maps


def kernel(**inputs):
    global _PROG
    from concourse.bass_utils import run_bass_kernel_spmd
    if _PROG is None:
        _PROG = _build()
    in_maps = _host_prep(inputs)
    res = run_bass_kernel_spmd(
        _PROG, in_maps, core_ids=list(range(NC)),
        trace=os.environ.get("KERNEL_TRACE") == "1")
    if res.exec_time_ns is not None:
        kernel.last_exec_time_ns = res.exec_time_ns
    preds = np.concatenate(
        [res.results[i]["preds"].reshape(T, B, VS) for i in range(NC)], axis=2)
    return np.ascontiguousarray(preds.transpose(1, 0, 2))
